# revision 12
# baseline (speedup 1.0000x reference)
"""Trainium2 Bass kernel for nn_CustomAttentionLayer (GQA attention + RoPE + o_proj).

Sharding: head-parallel, 8-way over (batch, kv-head-group): core c handles
batch c//4 and kv head g=c%4, i.e. query heads 4g..4g+3. Each core computes
q/k/v projections for exactly its heads (zero redundant compute), attention
over the full 2048x2048 score matrix for its 4 query heads, and a PARTIAL
output projection (contribution of its heads to the full [S,H] output).
The host sums the 4 partials per batch — no on-device collectives.

All matmul operands are bfloat16 (fp32 accumulate in PSUM): same TensorE
throughput as float32r in this regime but no small-free-dim penalty, half
the DMA traffic, and 2x DVE throughput for the softmax-denominator
accumulation. Measured end-to-end rel err ~8e-3 (budget 2e-2).

Schedule: the attention j-loop is software-pipelined (pv lags sc by one
iteration to hide the exp latency). The q-projection of the next chunk and
the o-projection of the previous chunk are interleaved into the j-loop as
TensorE filler at iters 2..14/15, with the first two matmuls of each filler
stream spilled into the previous group's tail so the group boundary (rsb ->
reciprocal -> PSUM-free chain) is covered with ready PE work. o_proj SBUF
copies are emitted ~3 iterations after their PSUM group closes so the
Activation stream (which must sustain one 570ns exp per iteration) never
head-of-line blocks on a PE matmul. Row sums come from a running DVE bf16
accumulation (2x rate) plus ONE ones-matmul per (chunk, head).
"""

import os
import numpy as np
import ml_dtypes

import concourse.bass as bass
import concourse.mybir as mybir
import concourse.tile as tile
from concourse import bacc
from concourse.bass_utils import run_bass_kernel_spmd

B, S, H = 2, 2048, 2048
NH, NKV, HD = 16, 4, 128
HQ = NH // NKV                # 4 query heads per core
NC = 8                        # cores
KT = H // 128                 # 16 contraction tiles over H
CH = 512                      # query-chunk width (PSUM bank limit)
NCH = S // CH                 # 4 chunks
SJ = S // 128                 # 16 key-position tiles
SCALE = 1.0 / float(np.sqrt(HD))

f32 = mybir.dt.float32
bf16 = mybir.dt.bfloat16
FP = mybir.ActivationFunctionType
ALU = mybir.AluOpType


def _body(nc, tc, t):
    hT, wq, wk, wv, wo, ccD, ssD, outD = (
        t["hT"], t["wq"], t["wk"], t["wv"], t["wo"], t["cc"], t["ss"], t["out"]
    )
    with tc.tile_pool(name="main", bufs=1) as main, tc.tile_pool(
        name="psum", bufs=1, space="PSUM"
    ) as pp:
        # ---------------- persistent SBUF tensors ---------------------------
        cc = main.tile([128, S], bf16, tag="cc", bufs=1)
        ss = main.tile([128, S], bf16, tag="ss", bufs=1)
        wk_s = main.tile([128, KT * 128], bf16, tag="wk", bufs=1)
        wv_s = main.tile([128, KT * 128], bf16, tag="wv", bufs=1)
        wq_s = main.tile([128, KT * HQ * 128], bf16, tag="wq", bufs=1)
        wo_s = main.tile([128, HQ * H], bf16, tag="wo", bufs=1)
        qts = [[main.tile([128, CH], bf16, tag="qt", bufs=HQ * NCH,
                          name=f"qt{m}_{c}") for c in range(NCH)]
               for m in range(HQ)]
        kt = main.tile([128, S], bf16, tag="kt", bufs=1)
        vts = [main.tile([128, HD], bf16, tag="v", bufs=SJ, name=f"v{i}")
               for i in range(SJ)]
        # h chunk c as one tile [128, k, 512]; 4 quarter-DMAs per chunk keep
        # HWDGE setup cost (665ns each) off the startup critical path.
        hc = [main.tile([128, KT * CH], bf16, tag="hc", bufs=NCH,
                        name=f"hc{c}") for c in range(NCH)]
        hT_r = hT.rearrange("(k p) n -> p k n", p=128)

        def hq_t(c, k):
            return hc[c][:, k * CH:(k + 1) * CH]

        def load_hc(c):
            for qtr in range(4):
                nc.sync.dma_start(
                    hc[c][:, qtr * 4 * CH:(qtr + 1) * 4 * CH].rearrange(
                        "p (k j) -> p k j", j=CH),
                    hT_r[:, qtr * 4:(qtr + 1) * 4, bass.ts(c, CH)],
                )

        # DMA issue order ~ consumption order so startup is never DMA-paced.
        nc.sync.dma_start(wk_s[:], wk)
        load_hc(0)
        nc.sync.dma_start(wv_s[:], wv)
        nc.sync.dma_start(cc[:], ccD)
        nc.sync.dma_start(ss[:], ssD)
        load_hc(1)
        load_hc(2)
        load_hc(3)
        for part in range(4):
            w = KT * HQ * 128 // 4
            nc.sync.dma_start(wq_s[:, part * w:(part + 1) * w],
                              wq[:, part * w:(part + 1) * w])
        nc.sync.dma_start(wo_s[:], wo)

        def rope(dst, ps, cols, pool):
            # x = [xr; xi] on partition halves, cc = [c; c], ss = [s; s]:
            #   dst[0:64]   = xr*c - xi*s
            #   dst[64:128] = xr*s + xi*c
            w = dst.shape[-1]
            t1 = pool.tile([64, w], f32, tag="ropeA", bufs=2, name="t1")
            t2 = pool.tile([64, w], f32, tag="ropeB", bufs=2, name="t2")
            nc.vector.tensor_tensor(t1[:], ps[64:128, :], ss[64:128, cols],
                                    op=ALU.mult)
            nc.vector.tensor_tensor(t2[:], ps[0:64, :], cc[0:64, cols],
                                    op=ALU.mult)
            nc.vector.tensor_sub(dst[0:64, :], t2[:], t1[:])
            nc.vector.tensor_tensor(t1[:], ps[0:64, :], ss[0:64, cols],
                                    op=ALU.mult)
            nc.vector.tensor_tensor(t2[:], ps[64:128, :], cc[64:128, cols],
                                    op=ALU.mult)
            nc.vector.tensor_add(dst[64:128, :], t2[:], t1[:])

        def qproj_mm(c, m, k, wp):
            """One contraction-tile matmul of q-projection (c, m)."""
            if k == 0:
                qproj_mm.ps = pp.tile([128, CH], f32, tag="psq", bufs=2,
                                      name="psq")
            nc.tensor.matmul(
                qproj_mm.ps[:],
                wq_s[:, k * HQ * 128 + m * 128: k * HQ * 128 + (m + 1) * 128],
                hq_t(c, k),
                start=(k == 0), stop=(k == KT - 1),
            )
            if k == KT - 1:
                rope(qts[m][c][:], qproj_mm.ps, bass.ts(c, CH), wp)

        with tc.tile_pool(name="work", bufs=1) as wp:
            # ------- k+v projections, interleaved per chunk -----------------
            for c in range(NCH):
                cols = bass.ts(c, CH)
                ps = pp.tile([128, CH], f32, tag="mm", bufs=2, name="psk")
                for k in range(KT):
                    nc.tensor.matmul(
                        ps[:], wk_s[:, bass.ts(k, 128)], hq_t(c, k),
                        start=(k == 0), stop=(k == KT - 1),
                    )
                rope(kt[:, cols], ps, cols, wp)
                pv4 = pp.tile([128, 4 * HD], f32, tag="po", bufs=2, name="psv")
                for sub in range(4):
                    for k in range(KT):
                        nc.tensor.matmul(
                            pv4[:, bass.ts(sub, HD)],
                            hc[c][:, k * CH + sub * 128: k * CH + (sub + 1) * 128],
                            wv_s[:, bass.ts(k, 128)],
                            start=(k == 0), stop=(k == KT - 1),
                        )
                for sub in range(4):
                    nc.scalar.copy(vts[c * 4 + sub][:], pv4[:, bass.ts(sub, HD)])

            # ---------------- q projection chunk 0 --------------------------
            for m in range(HQ):
                for k in range(KT):
                    qproj_mm(0, m, k, wp)

            # ---------------- attention + interleaved q/o-proj --------------
            ones_b = wp.tile([128, 128], bf16, tag="ones", bufs=1)
            nc.vector.memset(ones_b[:], 1.0)

            onorm = [[wp.tile([128, CH], bf16, tag="onorm", bufs=2 * HQ,
                              name=f"on{c}_{m}") for m in range(HQ)]
                     for c in range(NCH)]

            o_pend = {}   # n -> psum tile awaiting copy+DMA

            def oproj_mm(c, sub, i):
                """o-matmul #i (n=i//4, m=i%4) of subgroup (c, sub)."""
                n, m = divmod(i, 4)
                if m == 0:
                    o_pend[n] = pp.tile([128, CH], f32, tag="po", bufs=2,
                                        name="pso")
                nc.tensor.matmul(
                    o_pend[n][:],
                    onorm[c][m][:, bass.ts(sub, 128)],
                    wo_s[:, m * H + n * CH: m * H + (n + 1) * CH],
                    start=(m == 0), stop=(m == HQ - 1),
                )

            def oproj_copy(c, sub, n):
                o_s = wp.tile([128, CH], bf16, tag="osb", bufs=6, name="osb")
                nc.scalar.copy(o_s[:], o_pend.pop(n)[:])
                nc.sync.dma_start(
                    outD[bass.ts(c * 4 + sub, 128), bass.ts(n, CH)], o_s[:])

            # group g = c*4 + mi. Filler streams per group:
            #   qfill(g): q-proj of (c+1, mi)      (exists iff c < NCH-1)
            #   ofill(g): o-proj subgroup (c-1, mi) (exists iff c > 0)
            # with each stream's first two matmuls emitted at the previous
            # group's tail, and qfill's k=15 (+rope) at its own tail.
            def qfill_of(g):
                c, mi = divmod(g, HQ)
                return (c + 1, mi) if c < NCH - 1 else None

            def ofill_of(g):
                if g is None:
                    return None
                c, mi = divmod(g, HQ)
                if g <= NCH * HQ:
                    c, mi = divmod(g, HQ)
                    if c >= 1:
                        return (c - 1, mi)
                return None

            def osub_of(g):
                # linear o-subgroup index: groups 4..19 map to (c-1, sub)
                if g < HQ or g >= 5 * HQ:
                    return None
                return divmod(g, HQ)[0] - 1, divmod(g, HQ)[1]

            def group_tail(g):
                """Emit boundary spill-over: qfill k15+rope of group g, o#0/#1
                of group g+1's subgroup, q k0/k1 of group g+1's qfill, and the
                delayed copy of subgroup(g)'s last po group."""
                qf = qfill_of(g) if g >= 0 else None
                if qf is not None:
                    qproj_mm(qf[0], qf[1], SJ - 1, wp)
                osub = osub_of(g) if g >= 0 else None
                if osub is not None:
                    oproj_copy(osub[0], osub[1], 3)
                nosub = osub_of(g + 1)
                if nosub is not None:
                    oproj_mm(nosub[0], nosub[1], 0)
                    oproj_mm(nosub[0], nosub[1], 1)
                nqf = qfill_of(g + 1) if g + 1 < NCH * HQ else None
                if nqf is not None:
                    qproj_mm(nqf[0], nqf[1], 0, wp)
                    qproj_mm(nqf[0], nqf[1], 1, wp)

            def attn_group(g):
                c, mi = divmod(g, HQ)
                qf = qfill_of(g)
                osub = osub_of(g)
                pv = pp.tile([128, CH], f32, tag="acc", bufs=2, name="pspv")
                exs = [None] * SJ
                acc = None
                for j in range(SJ):
                    sc = pp.tile([128, CH], f32, tag="mm", bufs=2, name="pssc")
                    nc.tensor.matmul(
                        sc[:], kt[:, bass.ts(j, 128)], qts[mi][c][:],
                        start=True, stop=True,
                    )
                    ex = wp.tile([128, CH], bf16, tag="expt", bufs=8,
                                 name="ex")
                    exs[j] = ex
                    nc.scalar.activation(ex[:], sc[:], FP.Exp, scale=SCALE)
                    if j == 0:
                        acc = ex
                    else:
                        nacc = wp.tile([128, CH], bf16, tag="exacc", bufs=2,
                                       name="exacc")
                        nc.vector.tensor_add(nacc[:], acc[:], ex[:])
                        acc = nacc
                    if qf is not None and 2 <= j <= SJ - 2:
                        qproj_mm(qf[0], qf[1], j, wp)
                    if j >= 1:
                        nc.tensor.matmul(
                            pv[:], vts[j - 1][:], exs[j - 1][:],
                            start=(j == 1), stop=False,
                        )
                    if osub is not None and j >= 2:
                        oproj_mm(osub[0], osub[1], j)
                        if j in (6, 10, 14):
                            oproj_copy(osub[0], osub[1], (j - 6) // 4)
                nc.tensor.matmul(pv[:], vts[SJ - 1][:], exs[SJ - 1][:],
                                 start=False, stop=True)
                rsb = pp.tile([128, CH], f32, tag="mm", bufs=2, name="psrs")
                nc.tensor.matmul(rsb[:], ones_b[:], acc[:], start=True,
                                 stop=True)
                # recip/onorm are emitted BEFORE the tail so they precede the
                # next q-rope burst in the DVE stream (prompt PSUM release).
                recipb = wp.tile([128, CH], f32, tag="recipb", bufs=2,
                                 name="rc")
                with nc.allow_low_precision(reason="1/rowsum feeds bf16 mul"):
                    nc.vector.reciprocal(recipb[:], rsb[:])
                nc.vector.tensor_tensor(onorm[c][mi][:], pv[:], recipb[:],
                                        op=ALU.mult)
                group_tail(g)

            # startup counterpart of group_tail(-1): q k0/k1 of group 0's
            # qfill stream
            qproj_mm(1, 0, 0, wp)
            qproj_mm(1, 0, 1, wp)

            for g in range(NCH * HQ):
                attn_group(g)

            # ---------------- tail: o-proj of the last chunk ----------------
            for sub in range(HQ):
                first = 2 if sub == 0 else 0   # (c3, 0) #0/#1 spilled above
                for i in range(first, 16):
                    oproj_mm(NCH - 1, sub, i)
                    if i % 4 == 3:
                        oproj_copy(NCH - 1, sub, i // 4)


def build(reps=1):
    nc = bacc.Bacc("TRN2", target_bir_lowering=False, debug=False,
                   num_devices=NC)
    t = {
        "hT": nc.dram_tensor("hT", [H, S], bf16, kind="ExternalInput").ap(),
        "wq": nc.dram_tensor("wq", [128, KT * HQ * 128], bf16,
                             kind="ExternalInput").ap(),
        "wk": nc.dram_tensor("wk", [128, KT * 128], bf16,
                             kind="ExternalInput").ap(),
        "wv": nc.dram_tensor("wv", [128, KT * 128], bf16,
                             kind="ExternalInput").ap(),
        "wo": nc.dram_tensor("wo", [128, HQ * H], bf16,
                             kind="ExternalInput").ap(),
        "cc": nc.dram_tensor("cc", [128, S], bf16, kind="ExternalInput").ap(),
        "ss": nc.dram_tensor("ss", [128, S], bf16, kind="ExternalInput").ap(),
        "out": nc.dram_tensor("out", [S, H], bf16, kind="ExternalOutput").ap(),
    }
    with tile.TileContext(nc) as tc:
        for _ in range(reps):
            _body(nc, tc, t)
    nc.compile()
    return nc


# per-head rope permutation: [even dims, odd dims]
_RP = np.r_[np.arange(0, HD, 2), np.arange(1, HD, 2)]


def _bf(x):
    return np.ascontiguousarray(x).astype(ml_dtypes.bfloat16)


def prep_inputs(hidden_states, freqs_cos, freqs_sin, Wq, Wk, Wv, Wo):
    """Host-side layout prep -> list of 8 per-core input maps."""
    cosT = np.concatenate([freqs_cos.T, freqs_cos.T], 0)  # [128, S]
    sinT = np.concatenate([freqs_sin.T, freqs_sin.T], 0)
    cc = _bf(cosT)
    ss = _bf(sinT)
    wqT = Wq.T    # [H, H]
    wkT = Wk.T    # [H, 512]
    wvT = Wv.T
    woT = Wo.T    # [H, H]
    hT = [_bf(hidden_states[b].T) for b in range(B)]
    in_maps = []
    for c in range(NC):
        b, g = divmod(c, HQ)
        # wq: [p, k, m, j] -> [128, KT*HQ*128]; head cols rope-permuted
        cols = np.concatenate(
            [(4 * g + m) * HD + _RP for m in range(HQ)])      # [512]
        wq_p = wqT[:, cols].reshape(KT, 128, HQ * 128).transpose(1, 0, 2)
        wk_p = wkT[:, g * HD + _RP].reshape(KT, 128, 128).transpose(1, 0, 2)
        wv_p = wvT[:, g * HD: (g + 1) * HD].reshape(
            KT, 128, 128).transpose(1, 0, 2)
        wo_p = woT[4 * g * HD: (4 * g + 4) * HD, :].reshape(
            HQ, 128, H).transpose(1, 0, 2)
        in_maps.append({
            "hT": hT[b],
            "wq": _bf(wq_p.reshape(128, -1)),
            "wk": _bf(wk_p.reshape(128, -1)),
            "wv": _bf(wv_p.reshape(128, -1)),
            "wo": _bf(wo_p.reshape(128, -1)),
            "cc": cc, "ss": ss,
        })
    return in_maps


_CACHE = {}


def _get_nc(reps=1):
    if reps not in _CACHE:
        _CACHE[reps] = build(reps)
    return _CACHE[reps]


def kernel(hidden_states, freqs_cos, freqs_sin, Wq, Wk, Wv, Wo):
    in_maps = prep_inputs(
        np.asarray(hidden_states, np.float32),
        np.asarray(freqs_cos, np.float32),
        np.asarray(freqs_sin, np.float32),
        np.asarray(Wq, np.float32),
        np.asarray(Wk, np.float32),
        np.asarray(Wv, np.float32),
        np.asarray(Wo, np.float32),
    )
    nc = _get_nc(int(os.environ.get("KERNEL_REPS", "1")))
    res = run_bass_kernel_spmd(nc, in_maps, core_ids=list(range(NC)))
    out = np.zeros((B, S, H), np.float32)
    for c in range(NC):
        b = c // HQ
        out[b] += np.asarray(res.results[c]["out"], np.float32)
    return out


# revision 14
# speedup vs baseline: 1.0492x; 1.0492x over previous
"""Trainium2 Bass kernel for nn_CustomAttentionLayer (GQA attention + RoPE + o_proj).

Sharding: head-parallel, 8-way over (batch, kv-head-group): core c handles
batch c//4 and kv head g=c%4, i.e. query heads 4g..4g+3. Each core computes
q/k/v projections for exactly its heads (zero redundant compute), attention
over the full 2048x2048 score matrix for its 4 query heads, and a PARTIAL
output projection (contribution of its heads to the full [S,H] output).
The host sums the 4 partials per batch — no on-device collectives.

All matmul operands are bfloat16 (fp32 accumulate in PSUM): same TensorE
throughput as float32r in this regime but no small-free-dim penalty, half
the DMA traffic, and 2x DVE throughput for the softmax-denominator
accumulation. Measured end-to-end rel err ~8e-3 (budget 2e-2).

Schedule: the attention j-loop is software-pipelined (pv lags sc by one
iteration to hide the exp latency). The q-projection of the next chunk and
the o-projection of the previous chunk are interleaved into the j-loop as
TensorE filler at iters 2..14/15, with the first two matmuls of each filler
stream spilled into the previous group's tail so the group boundary (rsb ->
reciprocal -> PSUM-free chain) is covered with ready PE work. o_proj SBUF
copies are emitted ~3 iterations after their PSUM group closes so the
Activation stream (which must sustain one 570ns exp per iteration) never
head-of-line blocks on a PE matmul. Row sums come from a running DVE bf16
accumulation (2x rate) plus ONE ones-matmul per (chunk, head).
"""

import os
import numpy as np
import ml_dtypes

import concourse.bass as bass
import concourse.mybir as mybir
import concourse.tile as tile
from concourse import bacc
from concourse.bass_utils import run_bass_kernel_spmd

B, S, H = 2, 2048, 2048
NH, NKV, HD = 16, 4, 128
HQ = NH // NKV                # 4 query heads per core
NC = 8                        # cores
KT = H // 128                 # 16 contraction tiles over H
CH = 512                      # query-chunk width (PSUM bank limit)
NCH = S // CH                 # 4 chunks
SJ = S // 128                 # 16 key-position tiles
SCALE = 1.0 / float(np.sqrt(HD))

f32 = mybir.dt.float32
bf16 = mybir.dt.bfloat16
FP = mybir.ActivationFunctionType
ALU = mybir.AluOpType


def _body(nc, tc, t):
    hT, wq, wk, wv, wo, ccD, ssD, outD = (
        t["hT"], t["wq"], t["wk"], t["wv"], t["wo"], t["cc"], t["ss"], t["out"]
    )
    with tc.tile_pool(name="main", bufs=1) as main, tc.tile_pool(
        name="psum", bufs=1, space="PSUM"
    ) as pp:
        # ---------------- persistent SBUF tensors ---------------------------
        cc = main.tile([128, S], bf16, tag="cc", bufs=1)
        ss = main.tile([128, S], bf16, tag="ss", bufs=1)
        wk_s = main.tile([128, KT * 128], bf16, tag="wk", bufs=1)
        wv_s = main.tile([128, KT * 128], bf16, tag="wv", bufs=1)
        wq_s = main.tile([128, KT * HQ * 128], bf16, tag="wq", bufs=1)
        wo_s = main.tile([128, HQ * H], bf16, tag="wo", bufs=1)
        qts = [[main.tile([128, CH], bf16, tag="qt", bufs=HQ * NCH,
                          name=f"qt{m}_{c}") for c in range(NCH)]
               for m in range(HQ)]
        kt = main.tile([128, S], bf16, tag="kt", bufs=1)
        vts = [main.tile([128, HD], bf16, tag="v", bufs=SJ, name=f"v{i}")
               for i in range(SJ)]
        # h chunk c as one tile [128, k, 512]; 4 quarter-DMAs per chunk keep
        # HWDGE setup cost (665ns each) off the startup critical path.
        hc = [main.tile([128, KT * CH], bf16, tag="hc", bufs=NCH,
                        name=f"hc{c}") for c in range(NCH)]
        hT_r = hT.rearrange("(k p) n -> p k n", p=128)

        def hq_t(c, k):
            return hc[c][:, k * CH:(k + 1) * CH]

        def load_hc(c):
            for qtr in range(4):
                nc.sync.dma_start(
                    hc[c][:, qtr * 4 * CH:(qtr + 1) * 4 * CH].rearrange(
                        "p (k j) -> p k j", j=CH),
                    hT_r[:, qtr * 4:(qtr + 1) * 4, bass.ts(c, CH)],
                )

        # DMA issue order ~ consumption order so startup is never DMA-paced.
        nc.sync.dma_start(wk_s[:], wk)
        load_hc(0)
        nc.sync.dma_start(wv_s[:], wv)
        nc.sync.dma_start(cc[:], ccD)
        nc.sync.dma_start(ss[:], ssD)
        load_hc(1)
        load_hc(2)
        load_hc(3)
        for part in range(4):
            w = KT * HQ * 128 // 4
            nc.sync.dma_start(wq_s[:, part * w:(part + 1) * w],
                              wq[:, part * w:(part + 1) * w])
        nc.sync.dma_start(wo_s[:], wo)

        def rope(dst, ps, cols, pool):
            # x = [xr; xi] on partition halves, cc = [c; c], ss = [s; s]:
            #   dst[0:64]   = xr*c - xi*s
            #   dst[64:128] = xr*s + xi*c
            w = dst.shape[-1]
            t1 = pool.tile([64, w], f32, tag="ropeA", bufs=2, name="t1")
            t2 = pool.tile([64, w], f32, tag="ropeB", bufs=2, name="t2")
            nc.vector.tensor_tensor(t1[:], ps[64:128, :], ss[64:128, cols],
                                    op=ALU.mult)
            nc.vector.tensor_tensor(t2[:], ps[0:64, :], cc[0:64, cols],
                                    op=ALU.mult)
            nc.vector.tensor_sub(dst[0:64, :], t2[:], t1[:])
            nc.vector.tensor_tensor(t1[:], ps[0:64, :], ss[0:64, cols],
                                    op=ALU.mult)
            nc.vector.tensor_tensor(t2[:], ps[64:128, :], cc[64:128, cols],
                                    op=ALU.mult)
            nc.vector.tensor_add(dst[64:128, :], t2[:], t1[:])

        def qproj_mm(c, m, k, wp):
            """One contraction-tile matmul of q-projection (c, m)."""
            if k == 0:
                qproj_mm.ps = pp.tile([128, CH], f32, tag="psq", bufs=2,
                                      name="psq")
            nc.tensor.matmul(
                qproj_mm.ps[:],
                wq_s[:, k * HQ * 128 + m * 128: k * HQ * 128 + (m + 1) * 128],
                hq_t(c, k),
                start=(k == 0), stop=(k == KT - 1),
            )
            if k == KT - 1:
                rope(qts[m][c][:], qproj_mm.ps, bass.ts(c, CH), wp)

        with tc.tile_pool(name="work", bufs=1) as wp:
            # ------- k+v projections, interleaved per chunk -----------------
            for c in range(NCH):
                cols = bass.ts(c, CH)
                ps = pp.tile([128, CH], f32, tag="mm", bufs=2, name="psk")
                for k in range(KT):
                    nc.tensor.matmul(
                        ps[:], wk_s[:, bass.ts(k, 128)], hq_t(c, k),
                        start=(k == 0), stop=(k == KT - 1),
                    )
                rope(kt[:, cols], ps, cols, wp)
                pv4 = pp.tile([128, 4 * HD], f32, tag="po", bufs=2, name="psv")
                for sub in range(4):
                    for k in range(KT):
                        nc.tensor.matmul(
                            pv4[:, bass.ts(sub, HD)],
                            hc[c][:, k * CH + sub * 128: k * CH + (sub + 1) * 128],
                            wv_s[:, bass.ts(k, 128)],
                            start=(k == 0), stop=(k == KT - 1),
                        )
                for sub in range(4):
                    nc.scalar.copy(vts[c * 4 + sub][:], pv4[:, bass.ts(sub, HD)])

            # ---------------- q projection chunk 0 --------------------------
            for m in range(HQ):
                for k in range(KT):
                    qproj_mm(0, m, k, wp)

            # ---------------- attention + interleaved q/o-proj --------------
            ones_b = wp.tile([128, 128], bf16, tag="ones", bufs=1)
            nc.vector.memset(ones_b[:], 1.0)

            onorm = [[wp.tile([128, CH], bf16, tag="onorm", bufs=2 * HQ,
                              name=f"on{c}_{m}") for m in range(HQ)]
                     for c in range(NCH)]

            o_pend = {}   # n -> psum tile awaiting copy+DMA

            def oproj_mm(c, sub, i):
                """o-matmul #i (n=i//4, m=i%4) of subgroup (c, sub)."""
                n, m = divmod(i, 4)
                if m == 0:
                    o_pend[n] = pp.tile([128, CH], f32, tag="po", bufs=2,
                                        name="pso")
                nc.tensor.matmul(
                    o_pend[n][:],
                    onorm[c][m][:, bass.ts(sub, 128)],
                    wo_s[:, m * H + n * CH: m * H + (n + 1) * CH],
                    start=(m == 0), stop=(m == HQ - 1),
                )

            def oproj_copy(c, sub, n):
                o_s = wp.tile([128, CH], bf16, tag="osb", bufs=6, name="osb")
                # alternate copy engine so neither Act nor DVE saturates
                if n % 2 == 0:
                    nc.vector.tensor_copy(o_s[:], o_pend.pop(n)[:])
                else:
                    nc.scalar.copy(o_s[:], o_pend.pop(n)[:])
                nc.sync.dma_start(
                    outD[bass.ts(c * 4 + sub, 128), bass.ts(n, CH)], o_s[:])

            # group g = c*4 + mi. Filler streams per group:
            #   qfill(g): q-proj of (c+1, mi)      (exists iff c < NCH-1)
            #   ofill(g): o-proj subgroup (c-1, mi) (exists iff c > 0)
            # with each stream's first two matmuls emitted at the previous
            # group's tail, and qfill's k=15 (+rope) at its own tail.
            def qfill_of(g):
                c, mi = divmod(g, HQ)
                return (c + 1, mi) if c < NCH - 1 else None

            def ofill_of(g):
                if g is None:
                    return None
                c, mi = divmod(g, HQ)
                if g <= NCH * HQ:
                    c, mi = divmod(g, HQ)
                    if c >= 1:
                        return (c - 1, mi)
                return None

            def osub_of(g):
                # linear o-subgroup index: groups 4..19 map to (c-1, sub)
                if g < HQ or g >= 5 * HQ:
                    return None
                return divmod(g, HQ)[0] - 1, divmod(g, HQ)[1]

            def group_tail(g):
                """Emit boundary spill-over: qfill k15+rope of group g, o#0/#1
                of group g+1's subgroup, q k0/k1 of group g+1's qfill, and the
                delayed copy of subgroup(g)'s last po group."""
                qf = qfill_of(g) if g >= 0 else None
                if qf is not None:
                    qproj_mm(qf[0], qf[1], SJ - 1, wp)
                osub = osub_of(g) if g >= 0 else None
                if osub is not None:
                    oproj_copy(osub[0], osub[1], 3)
                nosub = osub_of(g + 1)
                if nosub is not None:
                    oproj_mm(nosub[0], nosub[1], 0)
                    oproj_mm(nosub[0], nosub[1], 1)
                nqf = qfill_of(g + 1) if g + 1 < NCH * HQ else None
                if nqf is not None:
                    qproj_mm(nqf[0], nqf[1], 0, wp)
                    qproj_mm(nqf[0], nqf[1], 1, wp)

            def attn_group(g):
                c, mi = divmod(g, HQ)
                qf = qfill_of(g)
                osub = osub_of(g)
                pv = pp.tile([128, CH], f32, tag="acc", bufs=2, name="pspv")
                exs = [None] * SJ
                # two independent running sums: first half on the (otherwise
                # idle) GpSimd engine, second half on DVE so the group-end
                # chain (exp15 -> add -> rsb -> recip) stays on fast engines
                accA = accB = None
                for j in range(SJ):
                    sc = pp.tile([128, CH], f32, tag="mm", bufs=2, name="pssc")
                    nc.tensor.matmul(
                        sc[:], kt[:, bass.ts(j, 128)], qts[mi][c][:],
                        start=True, stop=True,
                    )
                    ex = wp.tile([128, CH], bf16, tag="expt", bufs=8,
                                 name="ex")
                    exs[j] = ex
                    nc.scalar.activation(ex[:], sc[:], FP.Exp, scale=SCALE)
                    if j == 0:
                        accA = ex
                    elif j <= 7:
                        nacc = wp.tile([128, CH], bf16, tag="exaccA", bufs=2,
                                       name="exaccA")
                        nc.gpsimd.tensor_add(nacc[:], accA[:], ex[:])
                        accA = nacc
                    elif j == 8:
                        accB = ex
                    else:
                        nacc = wp.tile([128, CH], bf16, tag="exaccB", bufs=2,
                                       name="exaccB")
                        nc.vector.tensor_add(nacc[:], accB[:], ex[:])
                        accB = nacc
                    if qf is not None and 2 <= j <= SJ - 2:
                        qproj_mm(qf[0], qf[1], j, wp)
                    if j >= 1:
                        nc.tensor.matmul(
                            pv[:], vts[j - 1][:], exs[j - 1][:],
                            start=(j == 1), stop=False,
                        )
                    if osub is not None and j >= 2:
                        oproj_mm(osub[0], osub[1], j)
                        if j in (6, 10, 14):
                            oproj_copy(osub[0], osub[1], (j - 6) // 4)
                nc.tensor.matmul(pv[:], vts[SJ - 1][:], exs[SJ - 1][:],
                                 start=False, stop=True)
                rsb = pp.tile([128, CH], f32, tag="mm", bufs=2, name="psrs")
                nc.tensor.matmul(rsb[:], ones_b[:], accA[:], start=True,
                                 stop=False)
                nc.tensor.matmul(rsb[:], ones_b[:], accB[:], start=False,
                                 stop=True)
                # recip/onorm are emitted BEFORE the tail so they precede the
                # next q-rope burst in the DVE stream (prompt PSUM release).
                recipb = wp.tile([128, CH], f32, tag="recipb", bufs=2,
                                 name="rc")
                with nc.allow_low_precision(reason="1/rowsum feeds bf16 mul"):
                    nc.vector.reciprocal(recipb[:], rsb[:])
                nc.vector.tensor_tensor(onorm[c][mi][:], pv[:], recipb[:],
                                        op=ALU.mult)
                group_tail(g)

            # startup counterpart of group_tail(-1): q k0/k1 of group 0's
            # qfill stream
            qproj_mm(1, 0, 0, wp)
            qproj_mm(1, 0, 1, wp)

            for g in range(NCH * HQ):
                attn_group(g)

            # ---------------- tail: o-proj of the last chunk ----------------
            for sub in range(HQ):
                first = 2 if sub == 0 else 0   # (c3, 0) #0/#1 spilled above
                for i in range(first, 16):
                    oproj_mm(NCH - 1, sub, i)
                    if i % 4 == 3:
                        oproj_copy(NCH - 1, sub, i // 4)


def build(reps=1):
    nc = bacc.Bacc("TRN2", target_bir_lowering=False, debug=False,
                   num_devices=NC)
    t = {
        "hT": nc.dram_tensor("hT", [H, S], bf16, kind="ExternalInput").ap(),
        "wq": nc.dram_tensor("wq", [128, KT * HQ * 128], bf16,
                             kind="ExternalInput").ap(),
        "wk": nc.dram_tensor("wk", [128, KT * 128], bf16,
                             kind="ExternalInput").ap(),
        "wv": nc.dram_tensor("wv", [128, KT * 128], bf16,
                             kind="ExternalInput").ap(),
        "wo": nc.dram_tensor("wo", [128, HQ * H], bf16,
                             kind="ExternalInput").ap(),
        "cc": nc.dram_tensor("cc", [128, S], bf16, kind="ExternalInput").ap(),
        "ss": nc.dram_tensor("ss", [128, S], bf16, kind="ExternalInput").ap(),
        "out": nc.dram_tensor("out", [S, H], bf16, kind="ExternalOutput").ap(),
    }
    with tile.TileContext(nc) as tc:
        for _ in range(reps):
            _body(nc, tc, t)
    nc.compile()
    return nc


# per-head rope permutation: [even dims, odd dims]
_RP = np.r_[np.arange(0, HD, 2), np.arange(1, HD, 2)]


def _bf(x):
    return np.ascontiguousarray(x).astype(ml_dtypes.bfloat16)


def prep_inputs(hidden_states, freqs_cos, freqs_sin, Wq, Wk, Wv, Wo):
    """Host-side layout prep -> list of 8 per-core input maps."""
    cosT = np.concatenate([freqs_cos.T, freqs_cos.T], 0)  # [128, S]
    sinT = np.concatenate([freqs_sin.T, freqs_sin.T], 0)
    cc = _bf(cosT)
    ss = _bf(sinT)
    wqT = Wq.T    # [H, H]
    wkT = Wk.T    # [H, 512]
    wvT = Wv.T
    woT = Wo.T    # [H, H]
    hT = [_bf(hidden_states[b].T) for b in range(B)]
    in_maps = []
    for c in range(NC):
        b, g = divmod(c, HQ)
        # wq: [p, k, m, j] -> [128, KT*HQ*128]; head cols rope-permuted
        cols = np.concatenate(
            [(4 * g + m) * HD + _RP for m in range(HQ)])      # [512]
        wq_p = wqT[:, cols].reshape(KT, 128, HQ * 128).transpose(1, 0, 2)
        wk_p = wkT[:, g * HD + _RP].reshape(KT, 128, 128).transpose(1, 0, 2)
        wv_p = wvT[:, g * HD: (g + 1) * HD].reshape(
            KT, 128, 128).transpose(1, 0, 2)
        wo_p = woT[4 * g * HD: (4 * g + 4) * HD, :].reshape(
            HQ, 128, H).transpose(1, 0, 2)
        in_maps.append({
            "hT": hT[b],
            "wq": _bf(wq_p.reshape(128, -1)),
            "wk": _bf(wk_p.reshape(128, -1)),
            "wv": _bf(wv_p.reshape(128, -1)),
            "wo": _bf(wo_p.reshape(128, -1)),
            "cc": cc, "ss": ss,
        })
    return in_maps


_CACHE = {}


def _get_nc(reps=1):
    if reps not in _CACHE:
        _CACHE[reps] = build(reps)
    return _CACHE[reps]


def kernel(hidden_states, freqs_cos, freqs_sin, Wq, Wk, Wv, Wo):
    in_maps = prep_inputs(
        np.asarray(hidden_states, np.float32),
        np.asarray(freqs_cos, np.float32),
        np.asarray(freqs_sin, np.float32),
        np.asarray(Wq, np.float32),
        np.asarray(Wk, np.float32),
        np.asarray(Wv, np.float32),
        np.asarray(Wo, np.float32),
    )
    nc = _get_nc(int(os.environ.get("KERNEL_REPS", "1")))
    res = run_bass_kernel_spmd(nc, in_maps, core_ids=list(range(NC)))
    out = np.zeros((B, S, H), np.float32)
    for c in range(NC):
        b = c // HQ
        out[b] += np.asarray(res.results[c]["out"], np.float32)
    return out


# revision 17
# speedup vs baseline: 1.0551x; 1.0056x over previous
"""Trainium2 Bass kernel for nn_CustomAttentionLayer (GQA attention + RoPE + o_proj).

Sharding: head-parallel, 8-way over (batch, kv-head-group): core c handles
batch c//4 and kv head g=c%4, i.e. query heads 4g..4g+3. Each core computes
q/k/v projections for exactly its heads (zero redundant compute), attention
over the full 2048x2048 score matrix for its 4 query heads, and a PARTIAL
output projection (contribution of its heads to the full [S,H] output).
The host sums the 4 partials per batch — no on-device collectives.

All matmul operands are bfloat16 (fp32 accumulate in PSUM): same TensorE
throughput as float32r in this regime but no small-free-dim penalty, half
the DMA traffic, and 2x DVE throughput for the softmax-denominator
accumulation. Measured end-to-end rel err ~8e-3 (budget 2e-2).

Schedule: the attention j-loop is software-pipelined (pv lags sc by one
iteration to hide the exp latency). The q-projection of the next chunk and
the o-projection of the previous chunk are interleaved into the j-loop as
TensorE filler at iters 2..14/15, with the first two matmuls of each filler
stream spilled into the previous group's tail so the group boundary (rsb ->
reciprocal -> PSUM-free chain) is covered with ready PE work. o_proj SBUF
copies are emitted ~3 iterations after their PSUM group closes so the
Activation stream (which must sustain one 570ns exp per iteration) never
head-of-line blocks on a PE matmul. Row sums come from a running DVE bf16
accumulation (2x rate) plus ONE ones-matmul per (chunk, head).
"""

import os
import numpy as np
import ml_dtypes

import concourse.bass as bass
import concourse.mybir as mybir
import concourse.tile as tile
from concourse import bacc
from concourse.bass_utils import run_bass_kernel_spmd

B, S, H = 2, 2048, 2048
NH, NKV, HD = 16, 4, 128
HQ = NH // NKV                # 4 query heads per core
NC = 8                        # cores
KT = H // 128                 # 16 contraction tiles over H
CH = 512                      # query-chunk width (PSUM bank limit)
NCH = S // CH                 # 4 chunks
SJ = S // 128                 # 16 key-position tiles
SCALE = 1.0 / float(np.sqrt(HD))

f32 = mybir.dt.float32
bf16 = mybir.dt.bfloat16
FP = mybir.ActivationFunctionType
ALU = mybir.AluOpType


def _body(nc, tc, t):
    hT, wq, wk, wv, wo, ccD, ssD, outD = (
        t["hT"], t["wq"], t["wk"], t["wv"], t["wo"], t["cc"], t["ss"], t["out"]
    )
    with tc.tile_pool(name="main", bufs=1) as main, tc.tile_pool(
        name="psum", bufs=1, space="PSUM"
    ) as pp:
        # ---------------- persistent SBUF tensors ---------------------------
        cc = main.tile([128, S], bf16, tag="cc", bufs=1)
        ss = main.tile([128, S], bf16, tag="ss", bufs=1)
        wk_s = main.tile([128, KT * 128], bf16, tag="wk", bufs=1)
        wv_s = main.tile([128, KT * 128], bf16, tag="wv", bufs=1)
        wq_s = main.tile([128, KT * HQ * 128], bf16, tag="wq", bufs=1)
        wo_s = main.tile([128, HQ * H], bf16, tag="wo", bufs=1)
        qts = [[main.tile([128, CH], bf16, tag="qt", bufs=HQ * NCH,
                          name=f"qt{m}_{c}") for c in range(NCH)]
               for m in range(HQ)]
        kt = main.tile([128, S], bf16, tag="kt", bufs=1)
        vts = [main.tile([128, HD], bf16, tag="v", bufs=SJ, name=f"v{i}")
               for i in range(SJ)]
        # h chunk c as one tile [128, k, 512]; 4 quarter-DMAs per chunk keep
        # HWDGE setup cost (665ns each) off the startup critical path.
        hc = [main.tile([128, KT * CH], bf16, tag="hc", bufs=NCH,
                        name=f"hc{c}") for c in range(NCH)]
        hT_r = hT.rearrange("(k p) n -> p k n", p=128)

        def hq_t(c, k):
            return hc[c][:, k * CH:(k + 1) * CH]

        def load_hc(c):
            for qtr in range(4):
                nc.sync.dma_start(
                    hc[c][:, qtr * 4 * CH:(qtr + 1) * 4 * CH].rearrange(
                        "p (k j) -> p k j", j=CH),
                    hT_r[:, qtr * 4:(qtr + 1) * 4, bass.ts(c, CH)],
                )

        # DMA issue order ~ consumption order so startup is never DMA-paced.
        # First piece is tiny so the first k-proj matmul starts ASAP.
        nc.sync.dma_start(wk_s[:, :128], wk[:, :128])
        nc.sync.dma_start(wk_s[:, 128:], wk[:, 128:])
        load_hc(0)
        nc.sync.dma_start(wv_s[:], wv)
        load_hc(1)
        nc.sync.dma_start(cc[:], ccD)
        nc.sync.dma_start(ss[:], ssD)
        load_hc(2)
        wqw = KT * HQ * 128 // 4
        nc.sync.dma_start(wq_s[:, :wqw], wq[:, :wqw])
        load_hc(3)
        for part in range(1, 4):
            nc.sync.dma_start(wq_s[:, part * wqw:(part + 1) * wqw],
                              wq[:, part * wqw:(part + 1) * wqw])
        nc.sync.dma_start(wo_s[:], wo)

        def rope(dst, ps, cols, pool):
            # x = [xr; xi] on partition halves, cc = [c; c], ss = [s; s]:
            #   dst[0:64]   = xr*c - xi*s
            #   dst[64:128] = xr*s + xi*c
            w = dst.shape[-1]
            t1 = pool.tile([64, w], f32, tag="ropeA", bufs=2, name="t1")
            t2 = pool.tile([64, w], f32, tag="ropeB", bufs=2, name="t2")
            nc.vector.tensor_tensor(t1[:], ps[64:128, :], ss[64:128, cols],
                                    op=ALU.mult)
            nc.vector.tensor_tensor(t2[:], ps[0:64, :], cc[0:64, cols],
                                    op=ALU.mult)
            nc.vector.tensor_sub(dst[0:64, :], t2[:], t1[:])
            nc.vector.tensor_tensor(t1[:], ps[0:64, :], ss[0:64, cols],
                                    op=ALU.mult)
            nc.vector.tensor_tensor(t2[:], ps[64:128, :], cc[64:128, cols],
                                    op=ALU.mult)
            nc.vector.tensor_add(dst[64:128, :], t2[:], t1[:])

        def qproj_mm(c, m, k, wp):
            """One contraction-tile matmul of q-projection (c, m)."""
            if k == 0:
                qproj_mm.ps = pp.tile([128, CH], f32, tag="psq", bufs=2,
                                      name="psq")
            nc.tensor.matmul(
                qproj_mm.ps[:],
                wq_s[:, k * HQ * 128 + m * 128: k * HQ * 128 + (m + 1) * 128],
                hq_t(c, k),
                start=(k == 0), stop=(k == KT - 1),
            )
            if k == KT - 1:
                rope(qts[m][c][:], qproj_mm.ps, bass.ts(c, CH), wp)

        with tc.tile_pool(name="work", bufs=1) as wp:
            # ------- k+v projections, interleaved per chunk -----------------
            for c in range(NCH):
                cols = bass.ts(c, CH)
                ps = pp.tile([128, CH], f32, tag="mm", bufs=2, name="psk")
                for k in range(KT):
                    nc.tensor.matmul(
                        ps[:], wk_s[:, bass.ts(k, 128)], hq_t(c, k),
                        start=(k == 0), stop=(k == KT - 1),
                    )
                rope(kt[:, cols], ps, cols, wp)
                pv4 = pp.tile([128, 4 * HD], f32, tag="po", bufs=2, name="psv")
                for sub in range(4):
                    for k in range(KT):
                        nc.tensor.matmul(
                            pv4[:, bass.ts(sub, HD)],
                            hc[c][:, k * CH + sub * 128: k * CH + (sub + 1) * 128],
                            wv_s[:, bass.ts(k, 128)],
                            start=(k == 0), stop=(k == KT - 1),
                        )
                for sub in range(4):
                    nc.scalar.copy(vts[c * 4 + sub][:], pv4[:, bass.ts(sub, HD)])

            # ---------------- q projection chunk 0 --------------------------
            for m in range(HQ):
                for k in range(KT):
                    qproj_mm(0, m, k, wp)

            # ---------------- attention + interleaved q/o-proj --------------
            ones_b = wp.tile([128, 128], bf16, tag="ones", bufs=1)
            nc.vector.memset(ones_b[:], 1.0)

            onorm = [[wp.tile([128, CH], bf16, tag="onorm", bufs=2 * HQ,
                              name=f"on{c}_{m}") for m in range(HQ)]
                     for c in range(NCH)]

            o_pend = {}   # n -> psum tile awaiting copy+DMA

            def oproj_mm(c, sub, i):
                """o-matmul #i (n=i//4, m=i%4) of subgroup (c, sub)."""
                n, m = divmod(i, 4)
                if m == 0:
                    o_pend[n] = pp.tile([128, CH], f32, tag="po", bufs=2,
                                        name="pso")
                nc.tensor.matmul(
                    o_pend[n][:],
                    onorm[c][m][:, bass.ts(sub, 128)],
                    wo_s[:, m * H + n * CH: m * H + (n + 1) * CH],
                    start=(m == 0), stop=(m == HQ - 1),
                )

            def oproj_copy(c, sub, n, dve=False):
                o_s = wp.tile([128, CH], bf16, tag="osb", bufs=6, name="osb")
                # alternate copy engine so neither Act nor DVE saturates;
                # chunks copied during the last-chunk groups (no q filler,
                # Act-paced) go entirely to DVE, which has slack there.
                if dve or n % 2 == 0:
                    nc.vector.tensor_copy(o_s[:], o_pend.pop(n)[:])
                else:
                    nc.scalar.copy(o_s[:], o_pend.pop(n)[:])
                nc.sync.dma_start(
                    outD[bass.ts(c * 4 + sub, 128), bass.ts(n, CH)], o_s[:])

            # group g = c*4 + mi. Filler streams per group:
            #   qfill(g): q-proj of (c+1, mi)      (exists iff c < NCH-1)
            #   ofill(g): o-proj subgroup (c-1, mi) (exists iff c > 0)
            # with each stream's first two matmuls emitted at the previous
            # group's tail, and qfill's k=15 (+rope) at its own tail.
            def qfill_of(g):
                c, mi = divmod(g, HQ)
                return (c + 1, mi) if c < NCH - 1 else None

            def ofill_of(g):
                if g is None:
                    return None
                c, mi = divmod(g, HQ)
                if g <= NCH * HQ:
                    c, mi = divmod(g, HQ)
                    if c >= 1:
                        return (c - 1, mi)
                return None

            def osub_of(g):
                # linear o-subgroup index: groups 4..19 map to (c-1, sub)
                if g < HQ or g >= 5 * HQ:
                    return None
                return divmod(g, HQ)[0] - 1, divmod(g, HQ)[1]

            def group_tail(g):
                """Emit boundary spill-over: qfill k15+rope of group g, o#0/#1
                of group g+1's subgroup, q k0/k1 of group g+1's qfill, and the
                delayed copy of subgroup(g)'s last po group."""
                qf = qfill_of(g) if g >= 0 else None
                if qf is not None:
                    qproj_mm(qf[0], qf[1], SJ - 1, wp)
                osub = osub_of(g) if g >= 0 else None
                if osub is not None:
                    oproj_copy(osub[0], osub[1], 3, dve=(g // HQ == NCH - 1))
                nosub = osub_of(g + 1)
                if nosub is not None:
                    oproj_mm(nosub[0], nosub[1], 0)
                    oproj_mm(nosub[0], nosub[1], 1)
                nqf = qfill_of(g + 1) if g + 1 < NCH * HQ else None
                if nqf is not None:
                    qproj_mm(nqf[0], nqf[1], 0, wp)
                    qproj_mm(nqf[0], nqf[1], 1, wp)

            def attn_group(g):
                c, mi = divmod(g, HQ)
                qf = qfill_of(g)
                osub = osub_of(g)
                pv = pp.tile([128, CH], f32, tag="acc", bufs=2, name="pspv")
                exs = [None] * SJ
                # two independent running sums: first half on the (otherwise
                # idle) GpSimd engine, second half on DVE so the group-end
                # chain (exp15 -> add -> rsb -> recip) stays on fast engines
                accA = accB = None
                for j in range(SJ):
                    sc = pp.tile([128, CH], f32, tag="mm", bufs=2, name="pssc")
                    nc.tensor.matmul(
                        sc[:], kt[:, bass.ts(j, 128)], qts[mi][c][:],
                        start=True, stop=True,
                    )
                    ex = wp.tile([128, CH], bf16, tag="expt", bufs=8,
                                 name="ex")
                    exs[j] = ex
                    nc.scalar.activation(ex[:], sc[:], FP.Exp, scale=SCALE)
                    if j == 0:
                        accA = ex
                    elif j <= 7:
                        nacc = wp.tile([128, CH], bf16, tag="exaccA", bufs=2,
                                       name="exaccA")
                        nc.gpsimd.tensor_add(nacc[:], accA[:], ex[:])
                        accA = nacc
                    elif j == 8:
                        accB = ex
                    else:
                        nacc = wp.tile([128, CH], bf16, tag="exaccB", bufs=2,
                                       name="exaccB")
                        nc.vector.tensor_add(nacc[:], accB[:], ex[:])
                        accB = nacc
                    if qf is not None and 2 <= j <= SJ - 2:
                        qproj_mm(qf[0], qf[1], j, wp)
                    if j >= 1:
                        nc.tensor.matmul(
                            pv[:], vts[j - 1][:], exs[j - 1][:],
                            start=(j == 1), stop=False,
                        )
                    if osub is not None and j >= 2:
                        oproj_mm(osub[0], osub[1], j)
                        if j in (6, 10, 14):
                            oproj_copy(osub[0], osub[1], (j - 6) // 4,
                                       dve=(c == NCH - 1))
                nc.tensor.matmul(pv[:], vts[SJ - 1][:], exs[SJ - 1][:],
                                 start=False, stop=True)
                rsb = pp.tile([128, CH], f32, tag="mm", bufs=2, name="psrs")
                nc.tensor.matmul(rsb[:], ones_b[:], accA[:], start=True,
                                 stop=False)
                nc.tensor.matmul(rsb[:], ones_b[:], accB[:], start=False,
                                 stop=True)
                # recip/onorm are emitted BEFORE the tail so they precede the
                # next q-rope burst in the DVE stream (prompt PSUM release).
                recipb = wp.tile([128, CH], f32, tag="recipb", bufs=2,
                                 name="rc")
                with nc.allow_low_precision(reason="1/rowsum feeds bf16 mul"):
                    nc.vector.reciprocal(recipb[:], rsb[:])
                nc.vector.tensor_tensor(onorm[c][mi][:], pv[:], recipb[:],
                                        op=ALU.mult)
                group_tail(g)

            # startup counterpart of group_tail(-1): q k0/k1 of group 0's
            # qfill stream
            qproj_mm(1, 0, 0, wp)
            qproj_mm(1, 0, 1, wp)

            for g in range(NCH * HQ):
                attn_group(g)

            # ---------------- tail: o-proj of the last chunk ----------------
            for sub in range(HQ):
                first = 2 if sub == 0 else 0   # (c3, 0) #0/#1 spilled above
                for i in range(first, 16):
                    oproj_mm(NCH - 1, sub, i)
                    if i % 4 == 3:
                        oproj_copy(NCH - 1, sub, i // 4)


def build(reps=1):
    nc = bacc.Bacc("TRN2", target_bir_lowering=False, debug=False,
                   num_devices=NC)
    t = {
        "hT": nc.dram_tensor("hT", [H, S], bf16, kind="ExternalInput").ap(),
        "wq": nc.dram_tensor("wq", [128, KT * HQ * 128], bf16,
                             kind="ExternalInput").ap(),
        "wk": nc.dram_tensor("wk", [128, KT * 128], bf16,
                             kind="ExternalInput").ap(),
        "wv": nc.dram_tensor("wv", [128, KT * 128], bf16,
                             kind="ExternalInput").ap(),
        "wo": nc.dram_tensor("wo", [128, HQ * H], bf16,
                             kind="ExternalInput").ap(),
        "cc": nc.dram_tensor("cc", [128, S], bf16, kind="ExternalInput").ap(),
        "ss": nc.dram_tensor("ss", [128, S], bf16, kind="ExternalInput").ap(),
        "out": nc.dram_tensor("out", [S, H], bf16, kind="ExternalOutput").ap(),
    }
    with tile.TileContext(nc) as tc:
        for _ in range(reps):
            _body(nc, tc, t)
    nc.compile()
    return nc


# per-head rope permutation: [even dims, odd dims]
_RP = np.r_[np.arange(0, HD, 2), np.arange(1, HD, 2)]


def _bf(x):
    return np.ascontiguousarray(x).astype(ml_dtypes.bfloat16)


def prep_inputs(hidden_states, freqs_cos, freqs_sin, Wq, Wk, Wv, Wo):
    """Host-side layout prep -> list of 8 per-core input maps."""
    cosT = np.concatenate([freqs_cos.T, freqs_cos.T], 0)  # [128, S]
    sinT = np.concatenate([freqs_sin.T, freqs_sin.T], 0)
    cc = _bf(cosT)
    ss = _bf(sinT)
    wqT = Wq.T    # [H, H]
    wkT = Wk.T    # [H, 512]
    wvT = Wv.T
    woT = Wo.T    # [H, H]
    hT = [_bf(hidden_states[b].T) for b in range(B)]
    in_maps = []
    for c in range(NC):
        b, g = divmod(c, HQ)
        # wq: [p, k, m, j] -> [128, KT*HQ*128]; head cols rope-permuted
        cols = np.concatenate(
            [(4 * g + m) * HD + _RP for m in range(HQ)])      # [512]
        wq_p = wqT[:, cols].reshape(KT, 128, HQ * 128).transpose(1, 0, 2)
        wk_p = wkT[:, g * HD + _RP].reshape(KT, 128, 128).transpose(1, 0, 2)
        wv_p = wvT[:, g * HD: (g + 1) * HD].reshape(
            KT, 128, 128).transpose(1, 0, 2)
        wo_p = woT[4 * g * HD: (4 * g + 4) * HD, :].reshape(
            HQ, 128, H).transpose(1, 0, 2)
        in_maps.append({
            "hT": hT[b],
            "wq": _bf(wq_p.reshape(128, -1)),
            "wk": _bf(wk_p.reshape(128, -1)),
            "wv": _bf(wv_p.reshape(128, -1)),
            "wo": _bf(wo_p.reshape(128, -1)),
            "cc": cc, "ss": ss,
        })
    return in_maps


_CACHE = {}


def _get_nc(reps=1):
    if reps not in _CACHE:
        _CACHE[reps] = build(reps)
    return _CACHE[reps]


def kernel(hidden_states, freqs_cos, freqs_sin, Wq, Wk, Wv, Wo):
    in_maps = prep_inputs(
        np.asarray(hidden_states, np.float32),
        np.asarray(freqs_cos, np.float32),
        np.asarray(freqs_sin, np.float32),
        np.asarray(Wq, np.float32),
        np.asarray(Wk, np.float32),
        np.asarray(Wv, np.float32),
        np.asarray(Wo, np.float32),
    )
    nc = _get_nc(int(os.environ.get("KERNEL_REPS", "1")))
    res = run_bass_kernel_spmd(nc, in_maps, core_ids=list(range(NC)))
    out = np.zeros((B, S, H), np.float32)
    for c in range(NC):
        b = c // HQ
        out[b] += np.asarray(res.results[c]["out"], np.float32)
    return out


# revision 20
# speedup vs baseline: 1.0645x; 1.0089x over previous
"""Trainium2 Bass kernel for nn_CustomAttentionLayer (GQA attention + RoPE + o_proj).

Sharding: head-parallel, 8-way over (batch, kv-head-group): core c handles
batch c//4 and kv head g=c%4, i.e. query heads 4g..4g+3. Each core computes
q/k/v projections for exactly its heads (zero redundant compute), attention
over the full 2048x2048 score matrix for its 4 query heads, and a PARTIAL
output projection (contribution of its heads to the full [S,H] output).
The host sums the 4 partials per batch — no on-device collectives.

All matmul operands are bfloat16 (fp32 accumulate in PSUM): same TensorE
throughput as float32r in this regime but no small-free-dim penalty, half
the DMA traffic, and 2x DVE throughput for the softmax-denominator
accumulation. Measured end-to-end rel err ~8e-3 (budget 2e-2).

Schedule: the attention j-loop is software-pipelined (pv lags sc by one
iteration to hide the exp latency). The q-projection of the next chunk and
the o-projection of the previous chunk are interleaved into the j-loop as
TensorE filler at iters 2..14/15, with the first two matmuls of each filler
stream spilled into the previous group's tail so the group boundary (rsb ->
reciprocal -> PSUM-free chain) is covered with ready PE work. o_proj SBUF
copies are emitted ~3 iterations after their PSUM group closes so the
Activation stream (which must sustain one 570ns exp per iteration) never
head-of-line blocks on a PE matmul. Row sums come from a running DVE bf16
accumulation (2x rate) plus ONE ones-matmul per (chunk, head).
"""

import os
import numpy as np
import ml_dtypes

import concourse.bass as bass
import concourse.mybir as mybir
import concourse.tile as tile
from concourse import bacc
from concourse.bass_utils import run_bass_kernel_spmd

B, S, H = 2, 2048, 2048
NH, NKV, HD = 16, 4, 128
HQ = NH // NKV                # 4 query heads per core
NC = 8                        # cores
KT = H // 128                 # 16 contraction tiles over H
CH = 512                      # query-chunk width (PSUM bank limit)
NCH = S // CH                 # 4 chunks
SJ = S // 128                 # 16 key-position tiles
SCALE = 1.0 / float(np.sqrt(HD))

f32 = mybir.dt.float32
bf16 = mybir.dt.bfloat16
fp8 = mybir.dt.float8e4
FP = mybir.ActivationFunctionType
ALU = mybir.AluOpType
WS, HS = 64.0, 8.0            # host-side fp8 pre-scales (folded into exp/Wo)
NKP = KT // 2                 # 8 contraction-tile PAIRS (DoubleRow)
ESCALE = SCALE / float((WS * HS) ** 2)


def _body(nc, tc, t):
    wo, ccD, ssD, outD = t["wo"], t["cc"], t["ss"], t["out"]
    with tc.tile_pool(name="main", bufs=1) as main, tc.tile_pool(
        name="psum", bufs=1, space="PSUM"
    ) as pp:
        # ---------------- persistent SBUF tensors ---------------------------
        cc = main.tile([128, S], bf16, tag="cc", bufs=1)
        ss = main.tile([128, S], bf16, tag="ss", bufs=1)
        wk8 = [main.tile([128, KT * 128], fp8, tag="wk8", bufs=2,
                         name=f"wk8{x}") for x in "hl"]
        wv8 = [main.tile([128, KT * 128], fp8, tag="wv8", bufs=2,
                         name=f"wv8{x}") for x in "hl"]
        wq8 = [main.tile([128, KT * HQ * 128], fp8, tag="wq8", bufs=2,
                         name=f"wq8{x}") for x in "hl"]
        wo_s = main.tile([128, HQ * H], bf16, tag="wo", bufs=1)
        qts = [[main.tile([128, CH], bf16, tag="qt", bufs=HQ * NCH,
                          name=f"qt{m}_{c}") for c in range(NCH)]
               for m in range(HQ)]
        kt = main.tile([128, S], bf16, tag="kt", bufs=1)
        vts = [main.tile([128, HD], bf16, tag="v", bufs=SJ, name=f"v{i}")
               for i in range(SJ)]
        # h chunk c, hi/lo fp8, host-packed [p, kpair, i, j] -> [128, 8192];
        # quarter-DMAs keep HWDGE setup cost off the startup critical path.
        hc8 = [[main.tile([128, NKP * 2 * CH], fp8, tag=f"hc8{x}", bufs=NCH,
                          name=f"hc8{x}{c}") for c in range(NCH)]
               for x in range(2)]

        def h_pair(x, c, kp):
            return hc8[x][c][:, kp * 2 * CH:(kp + 1) * 2 * CH].rearrange(
                "p (i j) -> p i j", i=2)

        def w_pair(wt, x, kp, w):
            return wt[x][:, kp * 2 * w:(kp + 1) * 2 * w].rearrange(
                "p (i j) -> p i j", i=2)

        def load_hc(c):
            for x, src in ((0, t["h8h"]), (1, t["h8l"])):
                for qtr in range(4):
                    cw = NKP * 2 * CH // 4
                    nc.sync.dma_start(
                        hc8[x][c][:, qtr * cw:(qtr + 1) * cw],
                        src[:, c * NKP * 2 * CH + qtr * cw:
                            c * NKP * 2 * CH + (qtr + 1) * cw],
                    )

        # DMA issue order ~ consumption order so startup is never DMA-paced.
        nc.sync.dma_start(wk8[0][:], t["wk8h"])
        nc.sync.dma_start(wk8[1][:], t["wk8l"])
        load_hc(0)
        nc.sync.dma_start(wv8[0][:], t["wv8h"])
        nc.sync.dma_start(wv8[1][:], t["wv8l"])
        load_hc(1)
        nc.sync.dma_start(cc[:], ccD)
        nc.sync.dma_start(ss[:], ssD)
        load_hc(2)
        nc.sync.dma_start(wq8[0][:], t["wq8h"])
        load_hc(3)
        nc.sync.dma_start(wq8[1][:], t["wq8l"])
        nc.sync.dma_start(wo_s[:], wo)

        def rope(dst, ps, cols, pool):
            # x = [xr; xi] on partition halves, cc = [c; c], ss = [s; s]:
            #   dst[0:64]   = xr*c - xi*s
            #   dst[64:128] = xr*s + xi*c
            w = dst.shape[-1]
            t1 = pool.tile([64, w], f32, tag="ropeA", bufs=2, name="t1")
            t2 = pool.tile([64, w], f32, tag="ropeB", bufs=2, name="t2")
            nc.vector.tensor_tensor(t1[:], ps[64:128, :], ss[64:128, cols],
                                    op=ALU.mult)
            nc.vector.tensor_tensor(t2[:], ps[0:64, :], cc[0:64, cols],
                                    op=ALU.mult)
            nc.vector.tensor_sub(dst[0:64, :], t2[:], t1[:])
            nc.vector.tensor_tensor(t1[:], ps[0:64, :], ss[0:64, cols],
                                    op=ALU.mult)
            nc.vector.tensor_tensor(t2[:], ps[64:128, :], cc[64:128, cols],
                                    op=ALU.mult)
            nc.vector.tensor_add(dst[64:128, :], t2[:], t1[:])

        # 3-term compensated fp8 DoubleRow: (wh+wl)(hh+hl) ~ wh*hh + wh*hl
        # + wl*hh; term t, pair kp -> step = t*NKP + kp, 24 steps total.
        QSTEPS = 3 * NKP

        def qproj_mm(c, m, step, wp):
            """One DoubleRow matmul (1/24) of q-projection (c, m)."""
            t, kp = divmod(step, NKP)
            wx, hx = ((0, 0), (0, 1), (1, 0))[t]
            if step == 0:
                qproj_mm.ps = pp.tile([128, CH], f32, tag="psq", bufs=2,
                                      name="psq")
            nc.tensor.matmul(
                qproj_mm.ps[:],
                w_pair(wq8, wx, kp, HQ * 128)[:, :, bass.ts(m, 128)],
                h_pair(hx, c, kp),
                start=(step == 0), stop=(step == QSTEPS - 1),
                perf_mode=mybir.MatmulPerfMode.DoubleRow,
            )
            if step == QSTEPS - 1:
                rope(qts[m][c][:], qproj_mm.ps, bass.ts(c, CH), wp)

        with tc.tile_pool(name="work", bufs=1) as wp:
            # ------- k+v projections, interleaved per chunk -----------------
            TERMS = ((0, 0), (0, 1), (1, 0))
            for c in range(NCH):
                cols = bass.ts(c, CH)
                ps = pp.tile([128, CH], f32, tag="mm", bufs=2, name="psk")
                for st in range(QSTEPS):
                    t_, kp = divmod(st, NKP)
                    wx, hx = TERMS[t_]
                    nc.tensor.matmul(
                        ps[:], w_pair(wk8, wx, kp, 128), h_pair(hx, c, kp),
                        start=(st == 0), stop=(st == QSTEPS - 1),
                        perf_mode=mybir.MatmulPerfMode.DoubleRow,
                    )
                rope(kt[:, cols], ps, cols, wp)
                pv4 = pp.tile([128, 4 * HD], f32, tag="po", bufs=2, name="psv")
                for sub in range(4):
                    for st in range(QSTEPS):
                        t_, kp = divmod(st, NKP)
                        wx, hx = TERMS[t_]
                        nc.tensor.matmul(
                            pv4[:, bass.ts(sub, HD)],
                            h_pair(hx, c, kp)[:, :, bass.ts(sub, 128)],
                            w_pair(wv8, wx, kp, 128),
                            start=(st == 0), stop=(st == QSTEPS - 1),
                            perf_mode=mybir.MatmulPerfMode.DoubleRow,
                        )
                for sub in range(4):
                    nc.scalar.copy(vts[c * 4 + sub][:], pv4[:, bass.ts(sub, HD)])

            # ---------------- q projection chunk 0 --------------------------
            for m in range(HQ):
                for st in range(QSTEPS):
                    qproj_mm(0, m, st, wp)

            # ---------------- attention + interleaved q/o-proj --------------
            ones_b = wp.tile([128, 128], bf16, tag="ones", bufs=1)
            nc.vector.memset(ones_b[:], 1.0)

            onorm = [[wp.tile([128, CH], bf16, tag="onorm", bufs=2 * HQ,
                              name=f"on{c}_{m}") for m in range(HQ)]
                     for c in range(NCH)]

            o_pend = {}   # n -> psum tile awaiting copy+DMA

            def oproj_mm(c, sub, i):
                """o-matmul #i (n=i//4, m=i%4) of subgroup (c, sub)."""
                n, m = divmod(i, 4)
                if m == 0:
                    o_pend[n] = pp.tile([128, CH], f32, tag="po", bufs=2,
                                        name="pso")
                nc.tensor.matmul(
                    o_pend[n][:],
                    onorm[c][m][:, bass.ts(sub, 128)],
                    wo_s[:, m * H + n * CH: m * H + (n + 1) * CH],
                    start=(m == 0), stop=(m == HQ - 1),
                )

            def oproj_copy(c, sub, n, dve=False):
                o_s = wp.tile([128, CH], bf16, tag="osb", bufs=6, name="osb")
                # alternate copy engine so neither Act nor DVE saturates;
                # chunks copied during the last-chunk groups (no q filler,
                # Act-paced) go entirely to DVE, which has slack there.
                if dve or n % 2 == 0:
                    nc.vector.tensor_copy(o_s[:], o_pend.pop(n)[:])
                else:
                    nc.scalar.copy(o_s[:], o_pend.pop(n)[:])
                nc.sync.dma_start(
                    outD[bass.ts(c * 4 + sub, 128), bass.ts(n, CH)], o_s[:])

            # group g = c*4 + mi. Filler streams per group:
            #   qfill(g): q-proj of (c+1, mi)      (exists iff c < NCH-1)
            #   ofill(g): o-proj subgroup (c-1, mi) (exists iff c > 0)
            # with each stream's first two matmuls emitted at the previous
            # group's tail, and qfill's k=15 (+rope) at its own tail.
            def qfill_of(g):
                c, mi = divmod(g, HQ)
                return (c + 1, mi) if c < NCH - 1 else None

            def ofill_of(g):
                if g is None:
                    return None
                c, mi = divmod(g, HQ)
                if g <= NCH * HQ:
                    c, mi = divmod(g, HQ)
                    if c >= 1:
                        return (c - 1, mi)
                return None

            def osub_of(g):
                # linear o-subgroup index: groups 4..19 map to (c-1, sub)
                if g < HQ or g >= 5 * HQ:
                    return None
                return divmod(g, HQ)[0] - 1, divmod(g, HQ)[1]

            def group_tail(g):
                """Emit boundary spill-over: qfill k15+rope of group g, o#0/#1
                of group g+1's subgroup, q k0/k1 of group g+1's qfill, and the
                delayed copy of subgroup(g)'s last po group."""
                qf = qfill_of(g) if g >= 0 else None
                if qf is not None:
                    qproj_mm(qf[0], qf[1], QSTEPS - 2, wp)
                    qproj_mm(qf[0], qf[1], QSTEPS - 1, wp)
                osub = osub_of(g) if g >= 0 else None
                if osub is not None:
                    oproj_copy(osub[0], osub[1], 3, dve=(g // HQ == NCH - 1))
                nosub = osub_of(g + 1)
                if nosub is not None:
                    oproj_mm(nosub[0], nosub[1], 0)
                    oproj_mm(nosub[0], nosub[1], 1)
                nqf = qfill_of(g + 1) if g + 1 < NCH * HQ else None
                if nqf is not None:
                    qproj_mm(nqf[0], nqf[1], 0, wp)
                    qproj_mm(nqf[0], nqf[1], 1, wp)

            def attn_group(g):
                c, mi = divmod(g, HQ)
                qf = qfill_of(g)
                osub = osub_of(g)
                pv = pp.tile([128, CH], f32, tag="acc", bufs=2, name="pspv")
                exs = [None] * SJ
                # two independent running sums: first half on the (otherwise
                # idle) GpSimd engine, second half on DVE so the group-end
                # chain (exp15 -> add -> rsb -> recip) stays on fast engines
                accA = accB = None
                for j in range(SJ):
                    sc = pp.tile([128, CH], f32, tag="mm", bufs=2, name="pssc")
                    nc.tensor.matmul(
                        sc[:], kt[:, bass.ts(j, 128)], qts[mi][c][:],
                        start=True, stop=True,
                    )
                    ex = wp.tile([128, CH], bf16, tag="expt", bufs=8,
                                 name="ex")
                    exs[j] = ex
                    nc.scalar.activation(ex[:], sc[:], FP.Exp, scale=ESCALE)
                    if j == 0:
                        accA = ex
                    elif j <= 7:
                        nacc = wp.tile([128, CH], bf16, tag="exaccA", bufs=2,
                                       name="exaccA")
                        nc.gpsimd.tensor_add(nacc[:], accA[:], ex[:])
                        accA = nacc
                    elif j == 8:
                        accB = ex
                    else:
                        nacc = wp.tile([128, CH], bf16, tag="exaccB", bufs=2,
                                       name="exaccB")
                        nc.vector.tensor_add(nacc[:], accB[:], ex[:])
                        accB = nacc
                    if qf is not None and 2 <= j <= 11:
                        qproj_mm(qf[0], qf[1], 2 * j - 2, wp)
                        qproj_mm(qf[0], qf[1], 2 * j - 1, wp)
                    if j >= 1:
                        nc.tensor.matmul(
                            pv[:], vts[j - 1][:], exs[j - 1][:],
                            start=(j == 1), stop=False,
                        )
                    if osub is not None and j >= 2:
                        oproj_mm(osub[0], osub[1], j)
                        if j in (6, 10, 14):
                            oproj_copy(osub[0], osub[1], (j - 6) // 4,
                                       dve=(c == NCH - 1))
                nc.tensor.matmul(pv[:], vts[SJ - 1][:], exs[SJ - 1][:],
                                 start=False, stop=True)
                rsb = pp.tile([128, CH], f32, tag="mm", bufs=2, name="psrs")
                nc.tensor.matmul(rsb[:], ones_b[:], accA[:], start=True,
                                 stop=False)
                nc.tensor.matmul(rsb[:], ones_b[:], accB[:], start=False,
                                 stop=True)
                # recip/onorm are emitted BEFORE the tail so they precede the
                # next q-rope burst in the DVE stream (prompt PSUM release).
                recipb = wp.tile([128, CH], f32, tag="recipb", bufs=2,
                                 name="rc")
                with nc.allow_low_precision(reason="1/rowsum feeds bf16 mul"):
                    nc.vector.reciprocal(recipb[:], rsb[:])
                nc.vector.tensor_tensor(onorm[c][mi][:], pv[:], recipb[:],
                                        op=ALU.mult)
                group_tail(g)

            # startup counterpart of group_tail(-1): q k0/k1 of group 0's
            # qfill stream
            qproj_mm(1, 0, 0, wp)
            qproj_mm(1, 0, 1, wp)

            for g in range(NCH * HQ):
                attn_group(g)

            # ---------------- tail: o-proj of the last chunk ----------------
            for sub in range(HQ):
                first = 2 if sub == 0 else 0   # (c3, 0) #0/#1 spilled above
                for i in range(first, 16):
                    oproj_mm(NCH - 1, sub, i)
                    if i % 4 == 3:
                        oproj_copy(NCH - 1, sub, i // 4)


def build(reps=1):
    nc = bacc.Bacc("TRN2", target_bir_lowering=False, debug=False,
                   num_devices=NC)
    t = {
        "h8h": nc.dram_tensor("h8h", [128, NCH * NKP * 2 * CH], fp8,
                              kind="ExternalInput").ap(),
        "h8l": nc.dram_tensor("h8l", [128, NCH * NKP * 2 * CH], fp8,
                              kind="ExternalInput").ap(),
        "wq8h": nc.dram_tensor("wq8h", [128, KT * HQ * 128], fp8,
                               kind="ExternalInput").ap(),
        "wq8l": nc.dram_tensor("wq8l", [128, KT * HQ * 128], fp8,
                               kind="ExternalInput").ap(),
        "wk8h": nc.dram_tensor("wk8h", [128, KT * 128], fp8,
                               kind="ExternalInput").ap(),
        "wk8l": nc.dram_tensor("wk8l", [128, KT * 128], fp8,
                               kind="ExternalInput").ap(),
        "wv8h": nc.dram_tensor("wv8h", [128, KT * 128], fp8,
                               kind="ExternalInput").ap(),
        "wv8l": nc.dram_tensor("wv8l", [128, KT * 128], fp8,
                               kind="ExternalInput").ap(),
        "wo": nc.dram_tensor("wo", [128, HQ * H], bf16,
                             kind="ExternalInput").ap(),
        "cc": nc.dram_tensor("cc", [128, S], bf16, kind="ExternalInput").ap(),
        "ss": nc.dram_tensor("ss", [128, S], bf16, kind="ExternalInput").ap(),
        "out": nc.dram_tensor("out", [S, H], bf16, kind="ExternalOutput").ap(),
    }
    with tile.TileContext(nc) as tc:
        for _ in range(reps):
            _body(nc, tc, t)
    nc.compile()
    return nc


# per-head rope permutation: [even dims, odd dims]
_RP = np.r_[np.arange(0, HD, 2), np.arange(1, HD, 2)]
_E4 = ml_dtypes.float8_e4m3


def _bf(x):
    return np.ascontiguousarray(x).astype(ml_dtypes.bfloat16)


def _split8(x):
    """x -> (hi, lo) e4m3 with x ~ hi + lo."""
    hi = np.ascontiguousarray(x).astype(_E4)
    lo = (x - hi.astype(np.float32)).astype(_E4)
    return hi, lo


def _pack_h(x):
    # [2048 (=(2kp+i)*128+p), S] -> [p, c, kp, i, j] -> [128, NCH*NKP*2*CH]
    t = x.reshape(NKP, 2, 128, NCH, CH)
    return np.ascontiguousarray(
        t.transpose(2, 3, 0, 1, 4).reshape(128, -1))


def _pack_w(x):
    # [2048, M] -> [p, kp, i, M] -> [128, NKP*2*M]
    t = x.reshape(NKP, 2, 128, x.shape[1])
    return np.ascontiguousarray(t.transpose(2, 0, 1, 3).reshape(128, -1))


def prep_inputs(hidden_states, freqs_cos, freqs_sin, Wq, Wk, Wv, Wo):
    """Host-side layout prep -> list of 8 per-core input maps."""
    cosT = np.concatenate([freqs_cos.T, freqs_cos.T], 0)  # [128, S]
    sinT = np.concatenate([freqs_sin.T, freqs_sin.T], 0)
    cc = _bf(cosT)
    ss = _bf(sinT)
    wqT = Wq.T    # [H, H]
    wkT = Wk.T    # [H, 512]
    wvT = Wv.T
    woT = Wo.T    # [H, H]
    h8 = []
    for b in range(B):
        hi, lo = _split8(hidden_states[b].T * np.float32(HS))
        h8.append((_pack_h(hi), _pack_h(lo)))
    in_maps = []
    for c in range(NC):
        b, g = divmod(c, HQ)
        cols = np.concatenate(
            [(4 * g + m) * HD + _RP for m in range(HQ)])      # [512]
        wqh, wql = _split8(wqT[:, cols] * np.float32(WS))
        wkh, wkl = _split8(wkT[:, g * HD + _RP] * np.float32(WS))
        wvh, wvl = _split8(wvT[:, g * HD: (g + 1) * HD] * np.float32(WS))
        wo_p = (woT[4 * g * HD: (4 * g + 4) * HD, :] /
                np.float32(WS * HS)).reshape(HQ, 128, H).transpose(1, 0, 2)
        in_maps.append({
            "h8h": h8[b][0], "h8l": h8[b][1],
            "wq8h": _pack_w(wqh), "wq8l": _pack_w(wql),
            "wk8h": _pack_w(wkh), "wk8l": _pack_w(wkl),
            "wv8h": _pack_w(wvh), "wv8l": _pack_w(wvl),
            "wo": _bf(wo_p.reshape(128, -1)),
            "cc": cc, "ss": ss,
        })
    return in_maps


_CACHE = {}


def _get_nc(reps=1):
    if reps not in _CACHE:
        _CACHE[reps] = build(reps)
    return _CACHE[reps]


def kernel(hidden_states, freqs_cos, freqs_sin, Wq, Wk, Wv, Wo):
    in_maps = prep_inputs(
        np.asarray(hidden_states, np.float32),
        np.asarray(freqs_cos, np.float32),
        np.asarray(freqs_sin, np.float32),
        np.asarray(Wq, np.float32),
        np.asarray(Wk, np.float32),
        np.asarray(Wv, np.float32),
        np.asarray(Wo, np.float32),
    )
    nc = _get_nc(int(os.environ.get("KERNEL_REPS", "1")))
    res = run_bass_kernel_spmd(nc, in_maps, core_ids=list(range(NC)))
    out = np.zeros((B, S, H), np.float32)
    for c in range(NC):
        b = c // HQ
        out[b] += np.asarray(res.results[c]["out"], np.float32)
    return out


# revision 21
# speedup vs baseline: 1.0708x; 1.0059x over previous
"""Trainium2 Bass kernel for nn_CustomAttentionLayer (GQA attention + RoPE + o_proj).

Sharding: head-parallel, 8-way over (batch, kv-head-group): core c handles
batch c//4 and kv head g=c%4, i.e. query heads 4g..4g+3. Each core computes
q/k/v projections for exactly its heads (zero redundant compute), attention
over the full 2048x2048 score matrix for its 4 query heads, and a PARTIAL
output projection (contribution of its heads to the full [S,H] output).
The host sums the 4 partials per batch — no on-device collectives.

All matmul operands are bfloat16 (fp32 accumulate in PSUM): same TensorE
throughput as float32r in this regime but no small-free-dim penalty, half
the DMA traffic, and 2x DVE throughput for the softmax-denominator
accumulation. Measured end-to-end rel err ~8e-3 (budget 2e-2).

Schedule: the attention j-loop is software-pipelined (pv lags sc by one
iteration to hide the exp latency). The q-projection of the next chunk and
the o-projection of the previous chunk are interleaved into the j-loop as
TensorE filler at iters 2..14/15, with the first two matmuls of each filler
stream spilled into the previous group's tail so the group boundary (rsb ->
reciprocal -> PSUM-free chain) is covered with ready PE work. o_proj SBUF
copies are emitted ~3 iterations after their PSUM group closes so the
Activation stream (which must sustain one 570ns exp per iteration) never
head-of-line blocks on a PE matmul. Row sums come from a running DVE bf16
accumulation (2x rate) plus ONE ones-matmul per (chunk, head).
"""

import os
import numpy as np
import ml_dtypes

import concourse.bass as bass
import concourse.mybir as mybir
import concourse.tile as tile
from concourse import bacc
from concourse.bass_utils import run_bass_kernel_spmd

B, S, H = 2, 2048, 2048
NH, NKV, HD = 16, 4, 128
HQ = NH // NKV                # 4 query heads per core
NC = 8                        # cores
KT = H // 128                 # 16 contraction tiles over H
CH = 512                      # query-chunk width (PSUM bank limit)
NCH = S // CH                 # 4 chunks
SJ = S // 128                 # 16 key-position tiles
SCALE = 1.0 / float(np.sqrt(HD))

f32 = mybir.dt.float32
bf16 = mybir.dt.bfloat16
fp8 = mybir.dt.float8e4
FP = mybir.ActivationFunctionType
ALU = mybir.AluOpType
WS, HS = 64.0, 8.0            # host-side fp8 pre-scales (folded into exp/Wo)
NKP = KT // 2                 # 8 contraction-tile PAIRS (DoubleRow)
ESCALE = SCALE / float((WS * HS) ** 2)


def _body(nc, tc, t):
    wo, ccD, ssD, outD = t["wo"], t["cc"], t["ss"], t["out"]
    with tc.tile_pool(name="main", bufs=1) as main, tc.tile_pool(
        name="psum", bufs=1, space="PSUM"
    ) as pp:
        # ---------------- persistent SBUF tensors ---------------------------
        cc = main.tile([128, S], bf16, tag="cc", bufs=1)
        ss = main.tile([128, S], bf16, tag="ss", bufs=1)
        wk8 = [main.tile([128, KT * 128], fp8, tag="wk8", bufs=2,
                         name=f"wk8{x}") for x in "hl"]
        wv8 = [main.tile([128, KT * 128], fp8, tag="wv8", bufs=2,
                         name=f"wv8{x}") for x in "hl"]
        wq8 = [main.tile([128, KT * HQ * 128], fp8, tag="wq8", bufs=2,
                         name=f"wq8{x}") for x in "hl"]
        wo_s = main.tile([128, HQ * H], bf16, tag="wo", bufs=1)
        qts = [[main.tile([128, CH], bf16, tag="qt", bufs=HQ * NCH,
                          name=f"qt{m}_{c}") for c in range(NCH)]
               for m in range(HQ)]
        ktc = [main.tile([128, CH], bf16, tag="kt", bufs=NCH,
                         name=f"ktc{c}") for c in range(NCH)]
        vts = [main.tile([128, HD], bf16, tag="v", bufs=SJ, name=f"v{i}")
               for i in range(SJ)]
        # h chunk c, hi/lo fp8, host-packed [p, kpair, i, j] -> [128, 8192];
        # quarter-DMAs keep HWDGE setup cost off the startup critical path.
        hc8 = [[main.tile([128, NKP * 2 * CH], fp8, tag=f"hc8{x}", bufs=NCH,
                          name=f"hc8{x}{c}") for c in range(NCH)]
               for x in range(2)]

        def h_pair(x, c, kp):
            return hc8[x][c][:, kp * 2 * CH:(kp + 1) * 2 * CH].rearrange(
                "p (i j) -> p i j", i=2)

        def w_pair(wt, x, kp, w):
            return wt[x][:, kp * 2 * w:(kp + 1) * 2 * w].rearrange(
                "p (i j) -> p i j", i=2)

        def load_hc(c):
            for x, src in ((0, t["h8h"]), (1, t["h8l"])):
                for qtr in range(4):
                    cw = NKP * 2 * CH // 4
                    nc.sync.dma_start(
                        hc8[x][c][:, qtr * cw:(qtr + 1) * cw],
                        src[:, c * NKP * 2 * CH + qtr * cw:
                            c * NKP * 2 * CH + (qtr + 1) * cw],
                    )

        # DMA issue order ~ consumption order so startup is never DMA-paced.
        nc.sync.dma_start(wk8[0][:], t["wk8h"])
        nc.sync.dma_start(wk8[1][:], t["wk8l"])
        load_hc(0)
        nc.sync.dma_start(wv8[0][:], t["wv8h"])
        nc.sync.dma_start(wv8[1][:], t["wv8l"])
        load_hc(1)
        nc.sync.dma_start(cc[:], ccD)
        nc.sync.dma_start(ss[:], ssD)
        nc.sync.dma_start(wq8[0][:], t["wq8h"])
        nc.sync.dma_start(wq8[1][:], t["wq8l"])
        load_hc(2)
        load_hc(3)
        nc.sync.dma_start(wo_s[:], wo)

        def rope(dst, ps, cols, pool):
            # x = [xr; xi] on partition halves, cc = [c; c], ss = [s; s]:
            #   dst[0:64]   = xr*c - xi*s
            #   dst[64:128] = xr*s + xi*c
            w = dst.shape[-1]
            t1 = pool.tile([64, w], f32, tag="ropeA", bufs=2, name="t1")
            t2 = pool.tile([64, w], f32, tag="ropeB", bufs=2, name="t2")
            nc.vector.tensor_tensor(t1[:], ps[64:128, :], ss[64:128, cols],
                                    op=ALU.mult)
            nc.vector.tensor_tensor(t2[:], ps[0:64, :], cc[0:64, cols],
                                    op=ALU.mult)
            nc.vector.tensor_sub(dst[0:64, :], t2[:], t1[:])
            nc.vector.tensor_tensor(t1[:], ps[0:64, :], ss[0:64, cols],
                                    op=ALU.mult)
            nc.vector.tensor_tensor(t2[:], ps[64:128, :], cc[64:128, cols],
                                    op=ALU.mult)
            nc.vector.tensor_add(dst[64:128, :], t2[:], t1[:])

        # 3-term compensated fp8 DoubleRow: (wh+wl)(hh+hl) ~ wh*hh + wh*hl
        # + wl*hh; term t, pair kp -> step = t*NKP + kp, 24 steps total.
        QSTEPS = 3 * NKP

        def qproj_mm(c, m, step, wp):
            """One DoubleRow matmul (1/24) of q-projection (c, m)."""
            t, kp = divmod(step, NKP)
            wx, hx = ((0, 0), (0, 1), (1, 0))[t]
            if step == 0:
                qproj_mm.ps = pp.tile([128, CH], f32, tag="psq", bufs=2,
                                      name="psq")
            nc.tensor.matmul(
                qproj_mm.ps[:],
                w_pair(wq8, wx, kp, HQ * 128)[:, :, bass.ts(m, 128)],
                h_pair(hx, c, kp),
                start=(step == 0), stop=(step == QSTEPS - 1),
                perf_mode=mybir.MatmulPerfMode.DoubleRow,
            )
            if step == QSTEPS - 1:
                rope(qts[m][c][:], qproj_mm.ps, bass.ts(c, CH), wp)

        with tc.tile_pool(name="work", bufs=1) as wp:
            # ------- k+v projections, interleaved per chunk -----------------
            TERMS = ((0, 0), (0, 1), (1, 0))
            for c in range(NCH):
                cols = bass.ts(c, CH)
                ps = pp.tile([128, CH], f32, tag="mm", bufs=2, name="psk")
                for st in range(QSTEPS):
                    t_, kp = divmod(st, NKP)
                    wx, hx = TERMS[t_]
                    nc.tensor.matmul(
                        ps[:], w_pair(wk8, wx, kp, 128), h_pair(hx, c, kp),
                        start=(st == 0), stop=(st == QSTEPS - 1),
                        perf_mode=mybir.MatmulPerfMode.DoubleRow,
                    )
                rope(ktc[c][:], ps, cols, wp)
                pv4 = pp.tile([128, 4 * HD], f32, tag="po", bufs=2, name="psv")
                for sub in range(4):
                    for st in range(QSTEPS):
                        t_, kp = divmod(st, NKP)
                        wx, hx = TERMS[t_]
                        nc.tensor.matmul(
                            pv4[:, bass.ts(sub, HD)],
                            h_pair(hx, c, kp)[:, :, bass.ts(sub, 128)],
                            w_pair(wv8, wx, kp, 128),
                            start=(st == 0), stop=(st == QSTEPS - 1),
                            perf_mode=mybir.MatmulPerfMode.DoubleRow,
                        )
                for sub in range(4):
                    nc.scalar.copy(vts[c * 4 + sub][:], pv4[:, bass.ts(sub, HD)])

            # ---------------- q projection chunk 0 --------------------------
            for m in range(HQ):
                for st in range(QSTEPS):
                    qproj_mm(0, m, st, wp)

            # ---------------- attention + interleaved q/o-proj --------------
            ones_b = wp.tile([128, 128], bf16, tag="ones", bufs=1)
            nc.vector.memset(ones_b[:], 1.0)

            onorm = [[wp.tile([128, CH], bf16, tag="onorm", bufs=2 * HQ,
                              name=f"on{c}_{m}") for m in range(HQ)]
                     for c in range(NCH)]

            o_pend = {}   # n -> psum tile awaiting copy+DMA

            def oproj_mm(c, sub, i):
                """o-matmul #i (n=i//4, m=i%4) of subgroup (c, sub)."""
                n, m = divmod(i, 4)
                if m == 0:
                    o_pend[n] = pp.tile([128, CH], f32, tag="po", bufs=2,
                                        name="pso")
                nc.tensor.matmul(
                    o_pend[n][:],
                    onorm[c][m][:, bass.ts(sub, 128)],
                    wo_s[:, m * H + n * CH: m * H + (n + 1) * CH],
                    start=(m == 0), stop=(m == HQ - 1),
                )

            def oproj_copy(c, sub, n, dve=False):
                o_s = wp.tile([128, CH], bf16, tag="osb", bufs=6, name="osb")
                # alternate copy engine so neither Act nor DVE saturates;
                # chunks copied during the last-chunk groups (no q filler,
                # Act-paced) go entirely to DVE, which has slack there.
                if dve or n % 2 == 0:
                    nc.vector.tensor_copy(o_s[:], o_pend.pop(n)[:])
                else:
                    nc.scalar.copy(o_s[:], o_pend.pop(n)[:])
                nc.sync.dma_start(
                    outD[bass.ts(c * 4 + sub, 128), bass.ts(n, CH)], o_s[:])

            # group g = c*4 + mi. Filler streams per group:
            #   qfill(g): q-proj of (c+1, mi)      (exists iff c < NCH-1)
            #   ofill(g): o-proj subgroup (c-1, mi) (exists iff c > 0)
            # with each stream's first two matmuls emitted at the previous
            # group's tail, and qfill's k=15 (+rope) at its own tail.
            def qfill_of(g):
                c, mi = divmod(g, HQ)
                return (c + 1, mi) if c < NCH - 1 else None

            def ofill_of(g):
                if g is None:
                    return None
                c, mi = divmod(g, HQ)
                if g <= NCH * HQ:
                    c, mi = divmod(g, HQ)
                    if c >= 1:
                        return (c - 1, mi)
                return None

            def osub_of(g):
                # linear o-subgroup index: groups 4..19 map to (c-1, sub)
                if g < HQ or g >= 5 * HQ:
                    return None
                return divmod(g, HQ)[0] - 1, divmod(g, HQ)[1]

            def group_tail(g):
                """Emit boundary spill-over: qfill k15+rope of group g, o#0/#1
                of group g+1's subgroup, q k0/k1 of group g+1's qfill, and the
                delayed copy of subgroup(g)'s last po group."""
                qf = qfill_of(g) if g >= 0 else None
                if qf is not None:
                    qproj_mm(qf[0], qf[1], QSTEPS - 2, wp)
                    qproj_mm(qf[0], qf[1], QSTEPS - 1, wp)
                osub = osub_of(g) if g >= 0 else None
                if osub is not None:
                    oproj_copy(osub[0], osub[1], 3, dve=(g // HQ == NCH - 1))
                nosub = osub_of(g + 1)
                if nosub is not None:
                    oproj_mm(nosub[0], nosub[1], 0)
                    oproj_mm(nosub[0], nosub[1], 1)
                nqf = qfill_of(g + 1) if g + 1 < NCH * HQ else None
                if nqf is not None:
                    qproj_mm(nqf[0], nqf[1], 0, wp)
                    qproj_mm(nqf[0], nqf[1], 1, wp)

            def attn_group(g):
                c, mi = divmod(g, HQ)
                qf = qfill_of(g)
                osub = osub_of(g)
                pv = pp.tile([128, CH], f32, tag="acc", bufs=2, name="pspv")
                exs = [None] * SJ
                # two independent running sums: first half on the (otherwise
                # idle) GpSimd engine, second half on DVE so the group-end
                # chain (exp15 -> add -> rsb -> recip) stays on fast engines
                accA = accB = None
                for j in range(SJ):
                    sc = pp.tile([128, CH], f32, tag="mm", bufs=2, name="pssc")
                    nc.tensor.matmul(
                        sc[:], ktc[j // 4][:, bass.ts(j % 4, 128)],
                        qts[mi][c][:],
                        start=True, stop=True,
                    )
                    ex = wp.tile([128, CH], bf16, tag="expt", bufs=8,
                                 name="ex")
                    exs[j] = ex
                    nc.scalar.activation(ex[:], sc[:], FP.Exp, scale=ESCALE)
                    if j == 0:
                        accA = ex
                    elif j <= 7:
                        nacc = wp.tile([128, CH], bf16, tag="exaccA", bufs=2,
                                       name="exaccA")
                        nc.gpsimd.tensor_add(nacc[:], accA[:], ex[:])
                        accA = nacc
                    elif j == 8:
                        accB = ex
                    else:
                        nacc = wp.tile([128, CH], bf16, tag="exaccB", bufs=2,
                                       name="exaccB")
                        nc.vector.tensor_add(nacc[:], accB[:], ex[:])
                        accB = nacc
                    if qf is not None and 2 <= j <= 11:
                        qproj_mm(qf[0], qf[1], 2 * j - 2, wp)
                        qproj_mm(qf[0], qf[1], 2 * j - 1, wp)
                    if j >= 1:
                        nc.tensor.matmul(
                            pv[:], vts[j - 1][:], exs[j - 1][:],
                            start=(j == 1), stop=False,
                        )
                    if osub is not None and j >= 2:
                        oproj_mm(osub[0], osub[1], j)
                        if j in (6, 10, 14):
                            oproj_copy(osub[0], osub[1], (j - 6) // 4,
                                       dve=(c == NCH - 1))
                nc.tensor.matmul(pv[:], vts[SJ - 1][:], exs[SJ - 1][:],
                                 start=False, stop=True)
                rsb = pp.tile([128, CH], f32, tag="mm", bufs=2, name="psrs")
                nc.tensor.matmul(rsb[:], ones_b[:], accA[:], start=True,
                                 stop=False)
                nc.tensor.matmul(rsb[:], ones_b[:], accB[:], start=False,
                                 stop=True)
                # recip/onorm are emitted BEFORE the tail so they precede the
                # next q-rope burst in the DVE stream (prompt PSUM release).
                recipb = wp.tile([128, CH], f32, tag="recipb", bufs=2,
                                 name="rc")
                with nc.allow_low_precision(reason="1/rowsum feeds bf16 mul"):
                    nc.vector.reciprocal(recipb[:], rsb[:])
                nc.vector.tensor_tensor(onorm[c][mi][:], pv[:], recipb[:],
                                        op=ALU.mult)
                group_tail(g)

            # startup counterpart of group_tail(-1): q k0/k1 of group 0's
            # qfill stream
            qproj_mm(1, 0, 0, wp)
            qproj_mm(1, 0, 1, wp)

            for g in range(NCH * HQ):
                attn_group(g)

            # ---------------- tail: o-proj of the last chunk ----------------
            for sub in range(HQ):
                first = 2 if sub == 0 else 0   # (c3, 0) #0/#1 spilled above
                for i in range(first, 16):
                    oproj_mm(NCH - 1, sub, i)
                    if i % 4 == 3:
                        oproj_copy(NCH - 1, sub, i // 4)


def build(reps=1):
    nc = bacc.Bacc("TRN2", target_bir_lowering=False, debug=False,
                   num_devices=NC)
    t = {
        "h8h": nc.dram_tensor("h8h", [128, NCH * NKP * 2 * CH], fp8,
                              kind="ExternalInput").ap(),
        "h8l": nc.dram_tensor("h8l", [128, NCH * NKP * 2 * CH], fp8,
                              kind="ExternalInput").ap(),
        "wq8h": nc.dram_tensor("wq8h", [128, KT * HQ * 128], fp8,
                               kind="ExternalInput").ap(),
        "wq8l": nc.dram_tensor("wq8l", [128, KT * HQ * 128], fp8,
                               kind="ExternalInput").ap(),
        "wk8h": nc.dram_tensor("wk8h", [128, KT * 128], fp8,
                               kind="ExternalInput").ap(),
        "wk8l": nc.dram_tensor("wk8l", [128, KT * 128], fp8,
                               kind="ExternalInput").ap(),
        "wv8h": nc.dram_tensor("wv8h", [128, KT * 128], fp8,
                               kind="ExternalInput").ap(),
        "wv8l": nc.dram_tensor("wv8l", [128, KT * 128], fp8,
                               kind="ExternalInput").ap(),
        "wo": nc.dram_tensor("wo", [128, HQ * H], bf16,
                             kind="ExternalInput").ap(),
        "cc": nc.dram_tensor("cc", [128, S], bf16, kind="ExternalInput").ap(),
        "ss": nc.dram_tensor("ss", [128, S], bf16, kind="ExternalInput").ap(),
        "out": nc.dram_tensor("out", [S, H], bf16, kind="ExternalOutput").ap(),
    }
    with tile.TileContext(nc) as tc:
        for _ in range(reps):
            _body(nc, tc, t)
    nc.compile()
    return nc


# per-head rope permutation: [even dims, odd dims]
_RP = np.r_[np.arange(0, HD, 2), np.arange(1, HD, 2)]
_E4 = ml_dtypes.float8_e4m3


def _bf(x):
    return np.ascontiguousarray(x).astype(ml_dtypes.bfloat16)


def _split8(x):
    """x -> (hi, lo) e4m3 with x ~ hi + lo."""
    hi = np.ascontiguousarray(x).astype(_E4)
    lo = (x - hi.astype(np.float32)).astype(_E4)
    return hi, lo


def _pack_h(x):
    # [2048 (=(2kp+i)*128+p), S] -> [p, c, kp, i, j] -> [128, NCH*NKP*2*CH]
    t = x.reshape(NKP, 2, 128, NCH, CH)
    return np.ascontiguousarray(
        t.transpose(2, 3, 0, 1, 4).reshape(128, -1))


def _pack_w(x):
    # [2048, M] -> [p, kp, i, M] -> [128, NKP*2*M]
    t = x.reshape(NKP, 2, 128, x.shape[1])
    return np.ascontiguousarray(t.transpose(2, 0, 1, 3).reshape(128, -1))


def prep_inputs(hidden_states, freqs_cos, freqs_sin, Wq, Wk, Wv, Wo):
    """Host-side layout prep -> list of 8 per-core input maps."""
    cosT = np.concatenate([freqs_cos.T, freqs_cos.T], 0)  # [128, S]
    sinT = np.concatenate([freqs_sin.T, freqs_sin.T], 0)
    cc = _bf(cosT)
    ss = _bf(sinT)
    wqT = Wq.T    # [H, H]
    wkT = Wk.T    # [H, 512]
    wvT = Wv.T
    woT = Wo.T    # [H, H]
    h8 = []
    for b in range(B):
        hi, lo = _split8(hidden_states[b].T * np.float32(HS))
        h8.append((_pack_h(hi), _pack_h(lo)))
    in_maps = []
    for c in range(NC):
        b, g = divmod(c, HQ)
        cols = np.concatenate(
            [(4 * g + m) * HD + _RP for m in range(HQ)])      # [512]
        wqh, wql = _split8(wqT[:, cols] * np.float32(WS))
        wkh, wkl = _split8(wkT[:, g * HD + _RP] * np.float32(WS))
        wvh, wvl = _split8(wvT[:, g * HD: (g + 1) * HD] * np.float32(WS))
        wo_p = (woT[4 * g * HD: (4 * g + 4) * HD, :] /
                np.float32(WS * HS)).reshape(HQ, 128, H).transpose(1, 0, 2)
        in_maps.append({
            "h8h": h8[b][0], "h8l": h8[b][1],
            "wq8h": _pack_w(wqh), "wq8l": _pack_w(wql),
            "wk8h": _pack_w(wkh), "wk8l": _pack_w(wkl),
            "wv8h": _pack_w(wvh), "wv8l": _pack_w(wvl),
            "wo": _bf(wo_p.reshape(128, -1)),
            "cc": cc, "ss": ss,
        })
    return in_maps


_CACHE = {}


def _get_nc(reps=1):
    if reps not in _CACHE:
        _CACHE[reps] = build(reps)
    return _CACHE[reps]


def kernel(hidden_states, freqs_cos, freqs_sin, Wq, Wk, Wv, Wo):
    in_maps = prep_inputs(
        np.asarray(hidden_states, np.float32),
        np.asarray(freqs_cos, np.float32),
        np.asarray(freqs_sin, np.float32),
        np.asarray(Wq, np.float32),
        np.asarray(Wk, np.float32),
        np.asarray(Wv, np.float32),
        np.asarray(Wo, np.float32),
    )
    nc = _get_nc(int(os.environ.get("KERNEL_REPS", "1")))
    res = run_bass_kernel_spmd(nc, in_maps, core_ids=list(range(NC)))
    out = np.zeros((B, S, H), np.float32)
    for c in range(NC):
        b = c // HQ
        out[b] += np.asarray(res.results[c]["out"], np.float32)
    return out


# revision 24
# speedup vs baseline: 1.0758x; 1.0047x over previous
"""Trainium2 Bass kernel for nn_CustomAttentionLayer (GQA attention + RoPE + o_proj).

Sharding: head-parallel, 8-way over (batch, kv-head-group): core c handles
batch c//4 and kv head g=c%4, i.e. query heads 4g..4g+3. Each core computes
q/k/v projections for exactly its heads (zero redundant compute), attention
over the full 2048x2048 score matrix for its 4 query heads, and a PARTIAL
output projection (contribution of its heads to the full [S,H] output).
The host sums the 4 partials per batch — no on-device collectives.

All matmul operands are bfloat16 (fp32 accumulate in PSUM): same TensorE
throughput as float32r in this regime but no small-free-dim penalty, half
the DMA traffic, and 2x DVE throughput for the softmax-denominator
accumulation. Measured end-to-end rel err ~8e-3 (budget 2e-2).

Schedule: the attention j-loop is software-pipelined (pv lags sc by one
iteration to hide the exp latency). The q-projection of the next chunk and
the o-projection of the previous chunk are interleaved into the j-loop as
TensorE filler at iters 2..14/15, with the first two matmuls of each filler
stream spilled into the previous group's tail so the group boundary (rsb ->
reciprocal -> PSUM-free chain) is covered with ready PE work. o_proj SBUF
copies are emitted ~3 iterations after their PSUM group closes so the
Activation stream (which must sustain one 570ns exp per iteration) never
head-of-line blocks on a PE matmul. Row sums come from a running DVE bf16
accumulation (2x rate) plus ONE ones-matmul per (chunk, head).
"""

import os
import numpy as np
import ml_dtypes

import concourse.bass as bass
import concourse.mybir as mybir
import concourse.tile as tile
from concourse import bacc
from concourse.bass_utils import run_bass_kernel_spmd

B, S, H = 2, 2048, 2048
NH, NKV, HD = 16, 4, 128
HQ = NH // NKV                # 4 query heads per core
NC = 8                        # cores
KT = H // 128                 # 16 contraction tiles over H
CH = 512                      # query-chunk width (PSUM bank limit)
NCH = S // CH                 # 4 chunks
SJ = S // 128                 # 16 key-position tiles
SCALE = 1.0 / float(np.sqrt(HD))

f32 = mybir.dt.float32
bf16 = mybir.dt.bfloat16
fp8 = mybir.dt.float8e4
FP = mybir.ActivationFunctionType
ALU = mybir.AluOpType
WS, HS = 64.0, 8.0            # host-side fp8 pre-scales (folded into exp/Wo)
NKP = KT // 2                 # 8 contraction-tile PAIRS (DoubleRow)
ESCALE = SCALE / float((WS * HS) ** 2)


def _body(nc, tc, t):
    wo, ccD, ssD, outD = t["wo"], t["cc"], t["ss"], t["out"]
    with tc.tile_pool(name="main", bufs=1) as main, tc.tile_pool(
        name="psum", bufs=1, space="PSUM"
    ) as pp:
        # ---------------- persistent SBUF tensors ---------------------------
        cc = main.tile([128, S], bf16, tag="cc", bufs=1)
        ss = main.tile([128, S], bf16, tag="ss", bufs=1)
        wk8 = [main.tile([128, KT * 128], fp8, tag="wk8", bufs=2,
                         name=f"wk8{x}") for x in "hl"]
        wv8 = [main.tile([128, KT * 128], fp8, tag="wv8", bufs=2,
                         name=f"wv8{x}") for x in "hl"]
        wq8 = [[main.tile([128, KT * 128], fp8, tag="wq8", bufs=2 * HQ,
                          name=f"wq8{x}{m}") for m in range(HQ)]
               for x in "hl"]
        wo_s = main.tile([128, HQ * H], bf16, tag="wo", bufs=1)
        qts = [[main.tile([128, CH], bf16, tag="qt", bufs=HQ * NCH,
                          name=f"qt{m}_{c}") for c in range(NCH)]
               for m in range(HQ)]
        ktc = [main.tile([128, CH], bf16, tag="kt", bufs=NCH,
                         name=f"ktc{c}") for c in range(NCH)]
        vts = [main.tile([128, HD], bf16, tag="v", bufs=SJ, name=f"v{i}")
               for i in range(SJ)]
        # h chunk c, hi/lo fp8, host-packed [p, kpair, i, j] -> [128, 8192];
        # quarter-DMAs keep HWDGE setup cost off the startup critical path.
        hc8 = [[main.tile([128, NKP * 2 * CH], fp8, tag=f"hc8{x}", bufs=NCH,
                          name=f"hc8{x}{c}") for c in range(NCH)]
               for x in range(2)]

        def h_pair(x, c, kp):
            return hc8[x][c][:, kp * 2 * CH:(kp + 1) * 2 * CH].rearrange(
                "p (i j) -> p i j", i=2)

        def w_pair(wt, x, kp, w):
            return wt[x][:, kp * 2 * w:(kp + 1) * 2 * w].rearrange(
                "p (i j) -> p i j", i=2)

        def load_hc(c):
            for x, src in ((0, t["h8h"]), (1, t["h8l"])):
                for qtr in range(4):
                    cw = NKP * 2 * CH // 4
                    nc.sync.dma_start(
                        hc8[x][c][:, qtr * cw:(qtr + 1) * cw],
                        src[:, c * NKP * 2 * CH + qtr * cw:
                            c * NKP * 2 * CH + (qtr + 1) * cw],
                    )

        # DMA issue order ~ consumption order so startup is never DMA-paced.
        nc.sync.dma_start(wk8[0][:, :256], t["wk8h"][:, :256])
        nc.sync.dma_start(wk8[0][:, 256:], t["wk8h"][:, 256:])
        nc.sync.dma_start(wk8[1][:], t["wk8l"])
        load_hc(0)
        nc.sync.dma_start(wv8[0][:], t["wv8h"])
        nc.sync.dma_start(wv8[1][:], t["wv8l"])
        nc.sync.dma_start(cc[:], ccD)
        nc.sync.dma_start(ss[:], ssD)
        for m in range(HQ):
            nc.sync.dma_start(
                wq8[0][m][:], t["wq8h"][:, m * KT * 128:(m + 1) * KT * 128])
            nc.sync.dma_start(
                wq8[1][m][:], t["wq8l"][:, m * KT * 128:(m + 1) * KT * 128])
            if m < NCH - 1:
                load_hc(m + 1)
        nc.sync.dma_start(wo_s[:], wo)

        def rope(dst, ps, cols, pool):
            # x = [xr; xi] on partition halves, cc = [c; c], ss = [s; s]:
            #   dst[0:64]   = xr*c - xi*s
            #   dst[64:128] = xr*s + xi*c
            w = dst.shape[-1]
            t1 = pool.tile([64, w], f32, tag="ropeA", bufs=2, name="t1")
            t2 = pool.tile([64, w], f32, tag="ropeB", bufs=2, name="t2")
            nc.vector.tensor_tensor(t1[:], ps[64:128, :], ss[64:128, cols],
                                    op=ALU.mult)
            nc.vector.tensor_tensor(t2[:], ps[0:64, :], cc[0:64, cols],
                                    op=ALU.mult)
            nc.vector.tensor_sub(dst[0:64, :], t2[:], t1[:])
            nc.vector.tensor_tensor(t1[:], ps[0:64, :], ss[0:64, cols],
                                    op=ALU.mult)
            nc.vector.tensor_tensor(t2[:], ps[64:128, :], cc[64:128, cols],
                                    op=ALU.mult)
            nc.vector.tensor_add(dst[64:128, :], t2[:], t1[:])

        # 3-term compensated fp8 DoubleRow: (wh+wl)(hh+hl) ~ wh*hh + wh*hl
        # + wl*hh; term t, pair kp -> step = t*NKP + kp, 24 steps total.
        QSTEPS = 3 * NKP

        def qproj_mm(c, m, step, wp):
            """One DoubleRow matmul (1/24) of q-projection (c, m)."""
            t, kp = divmod(step, NKP)
            wx, hx = ((0, 0), (0, 1), (1, 0))[t]
            if step == 0:
                qproj_mm.ps = pp.tile([128, CH], f32, tag="psq", bufs=2,
                                      name="psq")
            nc.tensor.matmul(
                qproj_mm.ps[:],
                w_pair((wq8[0][m], wq8[1][m]), wx, kp, 128),
                h_pair(hx, c, kp),
                start=(step == 0), stop=(step == QSTEPS - 1),
                perf_mode=mybir.MatmulPerfMode.DoubleRow,
            )
            if step == QSTEPS - 1:
                rope(qts[m][c][:], qproj_mm.ps, bass.ts(c, CH), wp)

        with tc.tile_pool(name="work", bufs=1) as wp:
            # ------- k+v projections, interleaved per chunk -----------------
            TERMS = ((0, 0), (0, 1), (1, 0))
            for c in range(NCH):
                cols = bass.ts(c, CH)
                ps = pp.tile([128, CH], f32, tag="mm", bufs=2, name="psk")
                for st in range(QSTEPS):
                    t_, kp = divmod(st, NKP)
                    wx, hx = TERMS[t_]
                    nc.tensor.matmul(
                        ps[:], w_pair(wk8, wx, kp, 128), h_pair(hx, c, kp),
                        start=(st == 0), stop=(st == QSTEPS - 1),
                        perf_mode=mybir.MatmulPerfMode.DoubleRow,
                    )
                rope(ktc[c][:], ps, cols, wp)
                pv4 = pp.tile([128, 4 * HD], f32, tag="po", bufs=2, name="psv")
                for sub in range(4):
                    for st in range(QSTEPS):
                        t_, kp = divmod(st, NKP)
                        wx, hx = TERMS[t_]
                        nc.tensor.matmul(
                            pv4[:, bass.ts(sub, HD)],
                            h_pair(hx, c, kp)[:, :, bass.ts(sub, 128)],
                            w_pair(wv8, wx, kp, 128),
                            start=(st == 0), stop=(st == QSTEPS - 1),
                            perf_mode=mybir.MatmulPerfMode.DoubleRow,
                        )
                for sub in range(4):
                    nc.scalar.copy(vts[c * 4 + sub][:], pv4[:, bass.ts(sub, HD)])

            # ---------------- q projection chunk 0 --------------------------
            for m in range(HQ):
                for st in range(QSTEPS):
                    qproj_mm(0, m, st, wp)

            # ---------------- attention + interleaved q/o-proj --------------
            ones_b = wp.tile([128, 128], bf16, tag="ones", bufs=1)
            nc.vector.memset(ones_b[:], 1.0)

            onorm = [[wp.tile([128, CH], bf16, tag="onorm", bufs=2 * HQ,
                              name=f"on{c}_{m}") for m in range(HQ)]
                     for c in range(NCH)]

            o_pend = {}   # n -> psum tile awaiting copy+DMA

            def oproj_mm(c, sub, i):
                """o-matmul #i (n=i//4, m=i%4) of subgroup (c, sub)."""
                n, m = divmod(i, 4)
                if m == 0:
                    o_pend[n] = pp.tile([128, CH], f32, tag="po", bufs=2,
                                        name="pso")
                nc.tensor.matmul(
                    o_pend[n][:],
                    onorm[c][m][:, bass.ts(sub, 128)],
                    wo_s[:, m * H + n * CH: m * H + (n + 1) * CH],
                    start=(m == 0), stop=(m == HQ - 1),
                )

            def oproj_copy(c, sub, n, dve=False):
                o_s = wp.tile([128, CH], bf16, tag="osb", bufs=6, name="osb")
                # alternate copy engine so neither Act nor DVE saturates;
                # chunks copied during the last-chunk groups (no q filler,
                # Act-paced) go entirely to DVE, which has slack there.
                if dve or n % 2 == 0:
                    nc.vector.tensor_copy(o_s[:], o_pend.pop(n)[:])
                else:
                    nc.scalar.copy(o_s[:], o_pend.pop(n)[:])
                nc.sync.dma_start(
                    outD[bass.ts(c * 4 + sub, 128), bass.ts(n, CH)], o_s[:])

            # group g = c*4 + mi. Filler streams per group:
            #   qfill(g): q-proj of (c+1, mi)      (exists iff c < NCH-1)
            #   ofill(g): o-proj subgroup (c-1, mi) (exists iff c > 0)
            # with each stream's first two matmuls emitted at the previous
            # group's tail, and qfill's k=15 (+rope) at its own tail.
            def qfill_of(g):
                c, mi = divmod(g, HQ)
                return (c + 1, mi) if c < NCH - 1 else None

            def ofill_of(g):
                if g is None:
                    return None
                c, mi = divmod(g, HQ)
                if g <= NCH * HQ:
                    c, mi = divmod(g, HQ)
                    if c >= 1:
                        return (c - 1, mi)
                return None

            def osub_of(g):
                # linear o-subgroup index: groups 4..19 map to (c-1, sub)
                if g < HQ or g >= 5 * HQ:
                    return None
                return divmod(g, HQ)[0] - 1, divmod(g, HQ)[1]

            def group_tail(g):
                """Emit boundary spill-over: qfill k15+rope of group g, o#0/#1
                of group g+1's subgroup, q k0/k1 of group g+1's qfill, and the
                delayed copy of subgroup(g)'s last po group."""
                qf = qfill_of(g) if g >= 0 else None
                if qf is not None:
                    qproj_mm(qf[0], qf[1], QSTEPS - 2, wp)
                    qproj_mm(qf[0], qf[1], QSTEPS - 1, wp)
                osub = osub_of(g) if g >= 0 else None
                if osub is not None:
                    oproj_copy(osub[0], osub[1], 3, dve=(g // HQ == NCH - 1))
                nosub = osub_of(g + 1)
                if nosub is not None:
                    oproj_mm(nosub[0], nosub[1], 0)
                    oproj_mm(nosub[0], nosub[1], 1)
                nqf = qfill_of(g + 1) if g + 1 < NCH * HQ else None
                if nqf is not None:
                    qproj_mm(nqf[0], nqf[1], 0, wp)
                    qproj_mm(nqf[0], nqf[1], 1, wp)

            def attn_group(g):
                c, mi = divmod(g, HQ)
                qf = qfill_of(g)
                osub = osub_of(g)
                pv = pp.tile([128, CH], f32, tag="acc", bufs=2, name="pspv")
                exs = [None] * SJ
                # two independent running sums: first half on the (otherwise
                # idle) GpSimd engine, second half on DVE so the group-end
                # chain (exp15 -> add -> rsb -> recip) stays on fast engines
                accA = accB = None
                for j in range(SJ):
                    sc = pp.tile([128, CH], f32, tag="mm", bufs=2, name="pssc")
                    nc.tensor.matmul(
                        sc[:], ktc[j // 4][:, bass.ts(j % 4, 128)],
                        qts[mi][c][:],
                        start=True, stop=True,
                    )
                    ex = wp.tile([128, CH], bf16, tag="expt", bufs=8,
                                 name="ex")
                    exs[j] = ex
                    nc.scalar.activation(ex[:], sc[:], FP.Exp, scale=ESCALE)
                    if j == 0:
                        accA = ex
                    elif j <= 7:
                        nacc = wp.tile([128, CH], bf16, tag="exaccA", bufs=2,
                                       name="exaccA")
                        nc.gpsimd.tensor_add(nacc[:], accA[:], ex[:])
                        accA = nacc
                    elif j == 8:
                        accB = ex
                    else:
                        nacc = wp.tile([128, CH], bf16, tag="exaccB", bufs=2,
                                       name="exaccB")
                        nc.vector.tensor_add(nacc[:], accB[:], ex[:])
                        accB = nacc
                    if qf is not None and 2 <= j <= 11:
                        qproj_mm(qf[0], qf[1], 2 * j - 2, wp)
                        qproj_mm(qf[0], qf[1], 2 * j - 1, wp)
                    if j >= 1:
                        nc.tensor.matmul(
                            pv[:], vts[j - 1][:], exs[j - 1][:],
                            start=(j == 1), stop=False,
                        )
                    if osub is not None and j >= 2:
                        oproj_mm(osub[0], osub[1], j)
                        if j in (6, 10, 14):
                            oproj_copy(osub[0], osub[1], (j - 6) // 4,
                                       dve=(c == NCH - 1))
                nc.tensor.matmul(pv[:], vts[SJ - 1][:], exs[SJ - 1][:],
                                 start=False, stop=True)
                rsb = pp.tile([128, CH], f32, tag="mm", bufs=2, name="psrs")
                nc.tensor.matmul(rsb[:], ones_b[:], accA[:], start=True,
                                 stop=False)
                nc.tensor.matmul(rsb[:], ones_b[:], accB[:], start=False,
                                 stop=True)
                # recip/onorm are emitted BEFORE the tail so they precede the
                # next q-rope burst in the DVE stream (prompt PSUM release).
                recipb = wp.tile([128, CH], f32, tag="recipb", bufs=2,
                                 name="rc")
                with nc.allow_low_precision(reason="1/rowsum feeds bf16 mul"):
                    nc.vector.reciprocal(recipb[:], rsb[:])
                nc.vector.tensor_tensor(onorm[c][mi][:], pv[:], recipb[:],
                                        op=ALU.mult)
                group_tail(g)

            # startup counterpart of group_tail(-1): q k0/k1 of group 0's
            # qfill stream
            qproj_mm(1, 0, 0, wp)
            qproj_mm(1, 0, 1, wp)

            for g in range(NCH * HQ):
                attn_group(g)

            # ---------------- tail: o-proj of the last chunk ----------------
            for sub in range(HQ):
                first = 2 if sub == 0 else 0   # (c3, 0) #0/#1 spilled above
                for i in range(first, 16):
                    oproj_mm(NCH - 1, sub, i)
                    if i % 4 == 3:
                        oproj_copy(NCH - 1, sub, i // 4)


def build(reps=1):
    nc = bacc.Bacc("TRN2", target_bir_lowering=False, debug=False,
                   num_devices=NC)
    t = {
        "h8h": nc.dram_tensor("h8h", [128, NCH * NKP * 2 * CH], fp8,
                              kind="ExternalInput").ap(),
        "h8l": nc.dram_tensor("h8l", [128, NCH * NKP * 2 * CH], fp8,
                              kind="ExternalInput").ap(),
        "wq8h": nc.dram_tensor("wq8h", [128, KT * HQ * 128], fp8,
                               kind="ExternalInput").ap(),
        "wq8l": nc.dram_tensor("wq8l", [128, KT * HQ * 128], fp8,
                               kind="ExternalInput").ap(),
        "wk8h": nc.dram_tensor("wk8h", [128, KT * 128], fp8,
                               kind="ExternalInput").ap(),
        "wk8l": nc.dram_tensor("wk8l", [128, KT * 128], fp8,
                               kind="ExternalInput").ap(),
        "wv8h": nc.dram_tensor("wv8h", [128, KT * 128], fp8,
                               kind="ExternalInput").ap(),
        "wv8l": nc.dram_tensor("wv8l", [128, KT * 128], fp8,
                               kind="ExternalInput").ap(),
        "wo": nc.dram_tensor("wo", [128, HQ * H], bf16,
                             kind="ExternalInput").ap(),
        "cc": nc.dram_tensor("cc", [128, S], bf16, kind="ExternalInput").ap(),
        "ss": nc.dram_tensor("ss", [128, S], bf16, kind="ExternalInput").ap(),
        "out": nc.dram_tensor("out", [S, H], bf16, kind="ExternalOutput").ap(),
    }
    with tile.TileContext(nc) as tc:
        for _ in range(reps):
            _body(nc, tc, t)
    nc.compile()
    return nc


# per-head rope permutation: [even dims, odd dims]
_RP = np.r_[np.arange(0, HD, 2), np.arange(1, HD, 2)]
_E4 = ml_dtypes.float8_e4m3


def _bf(x):
    return np.ascontiguousarray(x).astype(ml_dtypes.bfloat16)


def _split8(x):
    """x -> (hi, lo) e4m3 with x ~ hi + lo."""
    hi = np.ascontiguousarray(x).astype(_E4)
    lo = (x - hi.astype(np.float32)).astype(_E4)
    return hi, lo


def _pack_h(x):
    # [2048 (=(2kp+i)*128+p), S] -> [p, c, kp, i, j] -> [128, NCH*NKP*2*CH]
    t = x.reshape(NKP, 2, 128, NCH, CH)
    return np.ascontiguousarray(
        t.transpose(2, 3, 0, 1, 4).reshape(128, -1))


def _pack_w(x):
    # [2048, M] -> [p, kp, i, M] -> [128, NKP*2*M]
    t = x.reshape(NKP, 2, 128, x.shape[1])
    return np.ascontiguousarray(t.transpose(2, 0, 1, 3).reshape(128, -1))


def prep_inputs(hidden_states, freqs_cos, freqs_sin, Wq, Wk, Wv, Wo):
    """Host-side layout prep -> list of 8 per-core input maps."""
    cosT = np.concatenate([freqs_cos.T, freqs_cos.T], 0)  # [128, S]
    sinT = np.concatenate([freqs_sin.T, freqs_sin.T], 0)
    cc = _bf(cosT)
    ss = _bf(sinT)
    wqT = Wq.T    # [H, H]
    wkT = Wk.T    # [H, 512]
    wvT = Wv.T
    woT = Wo.T    # [H, H]
    h8 = []
    for b in range(B):
        hi, lo = _split8(hidden_states[b].T * np.float32(HS))
        h8.append((_pack_h(hi), _pack_h(lo)))
    in_maps = []
    for c in range(NC):
        b, g = divmod(c, HQ)
        cols = np.concatenate(
            [(4 * g + m) * HD + _RP for m in range(HQ)])      # [512]
        wqh, wql = _split8(wqT[:, cols] * np.float32(WS))
        # head-major packing: [p, m, kp, i, j]
        wqh = np.concatenate(
            [_pack_w(wqh[:, m * 128:(m + 1) * 128]) for m in range(HQ)], 1)
        wql = np.concatenate(
            [_pack_w(wql[:, m * 128:(m + 1) * 128]) for m in range(HQ)], 1)
        wkh, wkl = _split8(wkT[:, g * HD + _RP] * np.float32(WS))
        wvh, wvl = _split8(wvT[:, g * HD: (g + 1) * HD] * np.float32(WS))
        wo_p = (woT[4 * g * HD: (4 * g + 4) * HD, :] /
                np.float32(WS * HS)).reshape(HQ, 128, H).transpose(1, 0, 2)
        in_maps.append({
            "h8h": h8[b][0], "h8l": h8[b][1],
            "wq8h": np.ascontiguousarray(wqh),
            "wq8l": np.ascontiguousarray(wql),
            "wk8h": _pack_w(wkh), "wk8l": _pack_w(wkl),
            "wv8h": _pack_w(wvh), "wv8l": _pack_w(wvl),
            "wo": _bf(wo_p.reshape(128, -1)),
            "cc": cc, "ss": ss,
        })
    return in_maps


_CACHE = {}


def _get_nc(reps=1):
    if reps not in _CACHE:
        _CACHE[reps] = build(reps)
    return _CACHE[reps]


def kernel(hidden_states, freqs_cos, freqs_sin, Wq, Wk, Wv, Wo):
    in_maps = prep_inputs(
        np.asarray(hidden_states, np.float32),
        np.asarray(freqs_cos, np.float32),
        np.asarray(freqs_sin, np.float32),
        np.asarray(Wq, np.float32),
        np.asarray(Wk, np.float32),
        np.asarray(Wv, np.float32),
        np.asarray(Wo, np.float32),
    )
    nc = _get_nc(int(os.environ.get("KERNEL_REPS", "1")))
    res = run_bass_kernel_spmd(nc, in_maps, core_ids=list(range(NC)))
    out = np.zeros((B, S, H), np.float32)
    for c in range(NC):
        b = c // HQ
        out[b] += np.asarray(res.results[c]["out"], np.float32)
    return out


# revision 27
# speedup vs baseline: 1.1288x; 1.0493x over previous
"""Trainium2 Bass kernel for nn_CustomAttentionLayer (GQA attention + RoPE + o_proj).

Sharding: head-parallel, 8-way over (batch, kv-head-group): core c handles
batch c//4 and kv head g=c%4, i.e. query heads 4g..4g+3. Each core computes
q/k/v projections for exactly its heads (zero redundant compute), attention
over the full 2048x2048 score matrix for its 4 query heads, and a PARTIAL
output projection (contribution of its heads to the full [S,H] output).
The host sums the 4 partials per batch — no on-device collectives.

All matmul operands are bfloat16 (fp32 accumulate in PSUM): same TensorE
throughput as float32r in this regime but no small-free-dim penalty, half
the DMA traffic, and 2x DVE throughput for the softmax-denominator
accumulation. Measured end-to-end rel err ~8e-3 (budget 2e-2).

Schedule: the attention j-loop is software-pipelined (pv lags sc by one
iteration to hide the exp latency). The q-projection of the next chunk and
the o-projection of the previous chunk are interleaved into the j-loop as
TensorE filler at iters 2..14/15, with the first two matmuls of each filler
stream spilled into the previous group's tail so the group boundary (rsb ->
reciprocal -> PSUM-free chain) is covered with ready PE work. o_proj SBUF
copies are emitted ~3 iterations after their PSUM group closes so the
Activation stream (which must sustain one 570ns exp per iteration) never
head-of-line blocks on a PE matmul. Row sums come from a running DVE bf16
accumulation (2x rate) plus ONE ones-matmul per (chunk, head).
"""

import os
import numpy as np
import ml_dtypes

import concourse.bass as bass
import concourse.mybir as mybir
import concourse.tile as tile
from concourse import bacc
from concourse.bass_utils import run_bass_kernel_spmd

B, S, H = 2, 2048, 2048
NH, NKV, HD = 16, 4, 128
HQ = NH // NKV                # 4 query heads per core
NC = 8                        # cores
KT = H // 128                 # 16 contraction tiles over H
CH = 512                      # query-chunk width (PSUM bank limit)
NCH = S // CH                 # 4 chunks
SJ = S // 128                 # 16 key-position tiles
SCALE = 1.0 / float(np.sqrt(HD))

f32 = mybir.dt.float32
bf16 = mybir.dt.bfloat16
fp8 = mybir.dt.float8e4
FP = mybir.ActivationFunctionType
ALU = mybir.AluOpType
WS, HS = 64.0, 8.0            # host-side fp8 pre-scales (folded into exp/Wo)
NKP = KT // 2                 # 8 contraction-tile PAIRS (DoubleRow)
ESCALE = SCALE / float((WS * HS) ** 2)


def _body(nc, tc, t):
    wo, ccD, ssD, outD = t["wo"], t["cc"], t["ss"], t["out"]
    with tc.tile_pool(name="main", bufs=1) as main, tc.tile_pool(
        name="psum", bufs=1, space="PSUM"
    ) as pp:
        # ---------------- persistent SBUF tensors ---------------------------
        cc = main.tile([128, S], bf16, tag="cc", bufs=1)
        ss = main.tile([128, S], bf16, tag="ss", bufs=1)
        wk8 = [main.tile([128, KT * 128], fp8, tag="wk8", bufs=2,
                         name=f"wk8{x}") for x in "hl"]
        wv8 = [main.tile([128, KT * 128], fp8, tag="wv8", bufs=2,
                         name=f"wv8{x}") for x in "hl"]
        wq8 = [[main.tile([128, KT * 128], fp8, tag="wq8", bufs=2 * HQ,
                          name=f"wq8{x}{m}") for m in range(HQ)]
               for x in "hl"]
        wo_s = main.tile([128, HQ * H], bf16, tag="wo", bufs=1)
        qts = [[main.tile([128, CH], bf16, tag="qt", bufs=HQ * NCH,
                          name=f"qt{m}_{c}") for c in range(NCH)]
               for m in range(HQ)]
        ktc = [main.tile([128, CH], bf16, tag="kt", bufs=NCH,
                         name=f"ktc{c}") for c in range(NCH)]
        vts = [main.tile([128, HD], bf16, tag="v", bufs=SJ, name=f"v{i}")
               for i in range(SJ)]
        # h chunk c, hi/lo fp8, host-packed [p, kpair, i, j] -> [128, 8192];
        # quarter-DMAs keep HWDGE setup cost off the startup critical path.
        hc8 = [[main.tile([128, NKP * 2 * CH], fp8, tag=f"hc8{x}", bufs=NCH,
                          name=f"hc8{x}{c}") for c in range(NCH)]
               for x in range(2)]

        def h_pair(x, c, kp):
            return hc8[x][c][:, kp * 2 * CH:(kp + 1) * 2 * CH].rearrange(
                "p (i j) -> p i j", i=2)

        def w_pair(wt, x, kp, w):
            return wt[x][:, kp * 2 * w:(kp + 1) * 2 * w].rearrange(
                "p (i j) -> p i j", i=2)

        def load_hc(c):
            for x, src in ((0, t["h8h"]), (1, t["h8l"])):
                for qtr in range(4):
                    cw = NKP * 2 * CH // 4
                    nc.sync.dma_start(
                        hc8[x][c][:, qtr * cw:(qtr + 1) * cw],
                        src[:, c * NKP * 2 * CH + qtr * cw:
                            c * NKP * 2 * CH + (qtr + 1) * cw],
                    )

        # DMA issue order ~ consumption order so startup is never DMA-paced.
        nc.sync.dma_start(wk8[0][:, :256], t["wk8h"][:, :256])
        nc.sync.dma_start(wk8[0][:, 256:], t["wk8h"][:, 256:])
        nc.sync.dma_start(wk8[1][:], t["wk8l"])
        load_hc(0)
        nc.sync.dma_start(wv8[0][:], t["wv8h"])
        nc.sync.dma_start(wv8[1][:], t["wv8l"])
        nc.sync.dma_start(cc[:], ccD)
        nc.sync.dma_start(ss[:], ssD)
        for m in range(HQ):
            nc.sync.dma_start(
                wq8[0][m][:], t["wq8h"][:, m * KT * 128:(m + 1) * KT * 128])
            nc.sync.dma_start(
                wq8[1][m][:], t["wq8l"][:, m * KT * 128:(m + 1) * KT * 128])
            if m < NCH - 1:
                load_hc(m + 1)
        nc.sync.dma_start(wo_s[:], wo)

        def rope(dst, ps, cols, pool):
            # x = [xr; xi] on partition halves, cc = [c; c], ss = [s; -s].
            # DVE op cost ~ free size only (partition count is free), so:
            #   t1 = ps*cc = (xr*c | xi*c)                   full-width
            #   t2[0:64]   = ps[64:]*ss[64:] = -xi*s         swapped placement
            #   t2[64:128] = ps[:64]*ss[:64] =  xr*s         swapped placement
            #   dst = t1 + t2                                full-width, bf16 2x
            # (inputs of each op share a partition offset — the lowering
            # rejects cross-offset inputs; out-vs-in offset is fine)
            w = dst.shape[-1]
            t1 = pool.tile([128, w], bf16, tag="ropeA", bufs=2, name="t1")
            t2 = pool.tile([128, w], bf16, tag="ropeB", bufs=2, name="t2")
            nc.vector.tensor_tensor(t1[:], ps[:, :], cc[:, cols], op=ALU.mult)
            nc.vector.tensor_tensor(t2[0:64, :], ps[64:128, :],
                                    ss[64:128, cols], op=ALU.mult)
            nc.vector.tensor_tensor(t2[64:128, :], ps[0:64, :],
                                    ss[0:64, cols], op=ALU.mult)
            nc.vector.tensor_add(dst[:, :], t1[:], t2[:])

        # 3-term compensated fp8 DoubleRow: (wh+wl)(hh+hl) ~ wh*hh + wh*hl
        # + wl*hh; term t, pair kp -> step = t*NKP + kp, 24 steps total.
        QSTEPS = 3 * NKP

        def qproj_mm(c, m, step, wp):
            """One DoubleRow matmul (1/24) of q-projection (c, m)."""
            t, kp = divmod(step, NKP)
            wx, hx = ((0, 0), (0, 1), (1, 0))[t]
            if step == 0:
                qproj_mm.ps = pp.tile([128, CH], f32, tag="psq", bufs=2,
                                      name="psq")
            nc.tensor.matmul(
                qproj_mm.ps[:],
                w_pair((wq8[0][m], wq8[1][m]), wx, kp, 128),
                h_pair(hx, c, kp),
                start=(step == 0), stop=(step == QSTEPS - 1),
                perf_mode=mybir.MatmulPerfMode.DoubleRow,
            )
            if step == QSTEPS - 1:
                rope(qts[m][c][:], qproj_mm.ps, bass.ts(c, CH), wp)

        with tc.tile_pool(name="work", bufs=1) as wp:
            # ------- k+v projections, interleaved per chunk -----------------
            TERMS = ((0, 0), (0, 1), (1, 0))
            for c in range(NCH):
                cols = bass.ts(c, CH)
                if c > 0:
                    # q-proj head c-1 interleaves with chunk c's k/v so its
                    # rope lands early in the DVE stream (attention g(c-1)
                    # can start as soon as its kt/q slices exist)
                    for st in range(QSTEPS):
                        qproj_mm(0, c - 1, st, wp)
                ps = pp.tile([128, CH], f32, tag="mm", bufs=2, name="psk")
                for st in range(QSTEPS):
                    t_, kp = divmod(st, NKP)
                    wx, hx = TERMS[t_]
                    nc.tensor.matmul(
                        ps[:], w_pair(wk8, wx, kp, 128), h_pair(hx, c, kp),
                        start=(st == 0), stop=(st == QSTEPS - 1),
                        perf_mode=mybir.MatmulPerfMode.DoubleRow,
                    )
                rope(ktc[c][:], ps, cols, wp)
                pv4 = pp.tile([128, 4 * HD], f32, tag="po", bufs=2, name="psv")
                for sub in range(4):
                    for st in range(QSTEPS):
                        t_, kp = divmod(st, NKP)
                        wx, hx = TERMS[t_]
                        nc.tensor.matmul(
                            pv4[:, bass.ts(sub, HD)],
                            h_pair(hx, c, kp)[:, :, bass.ts(sub, 128)],
                            w_pair(wv8, wx, kp, 128),
                            start=(st == 0), stop=(st == QSTEPS - 1),
                            perf_mode=mybir.MatmulPerfMode.DoubleRow,
                        )
                for sub in range(4):
                    nc.scalar.copy(vts[c * 4 + sub][:], pv4[:, bass.ts(sub, HD)])

            # ---------------- q projection chunk 0, last head ---------------
            for st in range(QSTEPS):
                qproj_mm(0, HQ - 1, st, wp)

            # ---------------- attention + interleaved q/o-proj --------------
            ones_b = wp.tile([128, 128], bf16, tag="ones", bufs=1)
            nc.vector.memset(ones_b[:], 1.0)

            onorm = [[wp.tile([128, CH], bf16, tag="onorm", bufs=2 * HQ,
                              name=f"on{c}_{m}") for m in range(HQ)]
                     for c in range(NCH)]

            o_pend = {}   # n -> psum tile awaiting copy+DMA

            def oproj_mm(c, sub, i):
                """o-matmul #i (n=i//4, m=i%4) of subgroup (c, sub)."""
                n, m = divmod(i, 4)
                if m == 0:
                    o_pend[n] = pp.tile([128, CH], f32, tag="po", bufs=2,
                                        name="pso")
                nc.tensor.matmul(
                    o_pend[n][:],
                    onorm[c][m][:, bass.ts(sub, 128)],
                    wo_s[:, m * H + n * CH: m * H + (n + 1) * CH],
                    start=(m == 0), stop=(m == HQ - 1),
                )

            def oproj_copy(c, sub, n, dve=False):
                o_s = wp.tile([128, CH], bf16, tag="osb", bufs=6, name="osb")
                # alternate copy engine so neither Act nor DVE saturates;
                # chunks copied during the last-chunk groups (no q filler,
                # Act-paced) go entirely to DVE, which has slack there.
                if dve or n % 2 == 0:
                    nc.vector.tensor_copy(o_s[:], o_pend.pop(n)[:])
                else:
                    nc.scalar.copy(o_s[:], o_pend.pop(n)[:])
                nc.sync.dma_start(
                    outD[bass.ts(c * 4 + sub, 128), bass.ts(n, CH)], o_s[:])

            # group g = c*4 + mi. Filler streams per group:
            #   qfill(g): q-proj of (c+1, mi)      (exists iff c < NCH-1)
            #   ofill(g): o-proj subgroup (c-1, mi) (exists iff c > 0)
            # with each stream's first two matmuls emitted at the previous
            # group's tail, and qfill's k=15 (+rope) at its own tail.
            def qfill_of(g):
                c, mi = divmod(g, HQ)
                return (c + 1, mi) if c < NCH - 1 else None

            def ofill_of(g):
                if g is None:
                    return None
                c, mi = divmod(g, HQ)
                if g <= NCH * HQ:
                    c, mi = divmod(g, HQ)
                    if c >= 1:
                        return (c - 1, mi)
                return None

            def osub_of(g):
                # linear o-subgroup index: groups 4..19 map to (c-1, sub)
                if g < HQ or g >= 5 * HQ:
                    return None
                return divmod(g, HQ)[0] - 1, divmod(g, HQ)[1]

            def group_tail(g):
                """Emit boundary spill-over: qfill k15+rope of group g, o#0/#1
                of group g+1's subgroup, q k0/k1 of group g+1's qfill, and the
                delayed copy of subgroup(g)'s last po group."""
                qf = qfill_of(g) if g >= 0 else None
                if qf is not None:
                    qproj_mm(qf[0], qf[1], QSTEPS - 2, wp)
                    qproj_mm(qf[0], qf[1], QSTEPS - 1, wp)
                osub = osub_of(g) if g >= 0 else None
                if osub is not None:
                    oproj_copy(osub[0], osub[1], 3, dve=(g // HQ == NCH - 1))
                nosub = osub_of(g + 1)
                if nosub is not None:
                    oproj_mm(nosub[0], nosub[1], 0)
                    oproj_mm(nosub[0], nosub[1], 1)
                nqf = qfill_of(g + 1) if g + 1 < NCH * HQ else None
                if nqf is not None:
                    qproj_mm(nqf[0], nqf[1], 0, wp)
                    qproj_mm(nqf[0], nqf[1], 1, wp)

            def attn_group(g):
                c, mi = divmod(g, HQ)
                qf = qfill_of(g)
                osub = osub_of(g)
                pv = pp.tile([128, CH], f32, tag="acc", bufs=2, name="pspv")
                exs = [None] * SJ
                # two independent running sums: first half on the (otherwise
                # idle) GpSimd engine, second half on DVE so the group-end
                # chain (exp15 -> add -> rsb -> recip) stays on fast engines
                accA = accB = None
                for j in range(SJ):
                    sc = pp.tile([128, CH], f32, tag="mm", bufs=2, name="pssc")
                    nc.tensor.matmul(
                        sc[:], ktc[j // 4][:, bass.ts(j % 4, 128)],
                        qts[mi][c][:],
                        start=True, stop=True,
                    )
                    ex = wp.tile([128, CH], bf16, tag="expt", bufs=8,
                                 name="ex")
                    exs[j] = ex
                    nc.scalar.activation(ex[:], sc[:], FP.Exp, scale=ESCALE)
                    if j == 0:
                        accA = ex
                    elif j <= 7:
                        nacc = wp.tile([128, CH], bf16, tag="exaccA", bufs=2,
                                       name="exaccA")
                        nc.gpsimd.tensor_add(nacc[:], accA[:], ex[:])
                        accA = nacc
                    elif j == 8:
                        accB = ex
                    else:
                        nacc = wp.tile([128, CH], bf16, tag="exaccB", bufs=2,
                                       name="exaccB")
                        nc.vector.tensor_add(nacc[:], accB[:], ex[:])
                        accB = nacc
                    if qf is not None and 2 <= j <= 11:
                        qproj_mm(qf[0], qf[1], 2 * j - 2, wp)
                        qproj_mm(qf[0], qf[1], 2 * j - 1, wp)
                    if j >= 1:
                        nc.tensor.matmul(
                            pv[:], vts[j - 1][:], exs[j - 1][:],
                            start=(j == 1), stop=False,
                        )
                    if osub is not None and j >= 2:
                        oproj_mm(osub[0], osub[1], j)
                        if j in (6, 10, 14):
                            oproj_copy(osub[0], osub[1], (j - 6) // 4,
                                       dve=(c == NCH - 1))
                nc.tensor.matmul(pv[:], vts[SJ - 1][:], exs[SJ - 1][:],
                                 start=False, stop=True)
                rsb = pp.tile([128, CH], f32, tag="mm", bufs=2, name="psrs")
                nc.tensor.matmul(rsb[:], ones_b[:], accA[:], start=True,
                                 stop=False)
                nc.tensor.matmul(rsb[:], ones_b[:], accB[:], start=False,
                                 stop=True)
                # recip/onorm are emitted BEFORE the tail so they precede the
                # next q-rope burst in the DVE stream (prompt PSUM release).
                recipb = wp.tile([128, CH], f32, tag="recipb", bufs=2,
                                 name="rc")
                with nc.allow_low_precision(reason="1/rowsum feeds bf16 mul"):
                    nc.vector.reciprocal(recipb[:], rsb[:])
                nc.vector.tensor_tensor(onorm[c][mi][:], pv[:], recipb[:],
                                        op=ALU.mult)
                group_tail(g)

            # startup counterpart of group_tail(-1): q k0/k1 of group 0's
            # qfill stream
            qproj_mm(1, 0, 0, wp)
            qproj_mm(1, 0, 1, wp)

            for g in range(NCH * HQ):
                attn_group(g)

            # ---------------- tail: o-proj of the last chunk ----------------
            for sub in range(HQ):
                first = 2 if sub == 0 else 0   # (c3, 0) #0/#1 spilled above
                for i in range(first, 16):
                    oproj_mm(NCH - 1, sub, i)
                    if i % 4 == 3:
                        oproj_copy(NCH - 1, sub, i // 4)


def build(reps=1):
    nc = bacc.Bacc("TRN2", target_bir_lowering=False, debug=False,
                   num_devices=NC)
    t = {
        "h8h": nc.dram_tensor("h8h", [128, NCH * NKP * 2 * CH], fp8,
                              kind="ExternalInput").ap(),
        "h8l": nc.dram_tensor("h8l", [128, NCH * NKP * 2 * CH], fp8,
                              kind="ExternalInput").ap(),
        "wq8h": nc.dram_tensor("wq8h", [128, KT * HQ * 128], fp8,
                               kind="ExternalInput").ap(),
        "wq8l": nc.dram_tensor("wq8l", [128, KT * HQ * 128], fp8,
                               kind="ExternalInput").ap(),
        "wk8h": nc.dram_tensor("wk8h", [128, KT * 128], fp8,
                               kind="ExternalInput").ap(),
        "wk8l": nc.dram_tensor("wk8l", [128, KT * 128], fp8,
                               kind="ExternalInput").ap(),
        "wv8h": nc.dram_tensor("wv8h", [128, KT * 128], fp8,
                               kind="ExternalInput").ap(),
        "wv8l": nc.dram_tensor("wv8l", [128, KT * 128], fp8,
                               kind="ExternalInput").ap(),
        "wo": nc.dram_tensor("wo", [128, HQ * H], bf16,
                             kind="ExternalInput").ap(),
        "cc": nc.dram_tensor("cc", [128, S], bf16, kind="ExternalInput").ap(),
        "ss": nc.dram_tensor("ss", [128, S], bf16, kind="ExternalInput").ap(),
        "out": nc.dram_tensor("out", [S, H], bf16, kind="ExternalOutput").ap(),
    }
    with tile.TileContext(nc) as tc:
        for _ in range(reps):
            _body(nc, tc, t)
    nc.compile()
    return nc


# per-head rope permutation: [even dims, odd dims]
_RP = np.r_[np.arange(0, HD, 2), np.arange(1, HD, 2)]
_E4 = ml_dtypes.float8_e4m3


def _bf(x):
    return np.ascontiguousarray(x).astype(ml_dtypes.bfloat16)


def _split8(x):
    """x -> (hi, lo) e4m3 with x ~ hi + lo."""
    hi = np.ascontiguousarray(x).astype(_E4)
    lo = (x - hi.astype(np.float32)).astype(_E4)
    return hi, lo


def _pack_h(x):
    # [2048 (=(2kp+i)*128+p), S] -> [p, c, kp, i, j] -> [128, NCH*NKP*2*CH]
    t = x.reshape(NKP, 2, 128, NCH, CH)
    return np.ascontiguousarray(
        t.transpose(2, 3, 0, 1, 4).reshape(128, -1))


def _pack_w(x):
    # [2048, M] -> [p, kp, i, M] -> [128, NKP*2*M]
    t = x.reshape(NKP, 2, 128, x.shape[1])
    return np.ascontiguousarray(t.transpose(2, 0, 1, 3).reshape(128, -1))


def prep_inputs(hidden_states, freqs_cos, freqs_sin, Wq, Wk, Wv, Wo):
    """Host-side layout prep -> list of 8 per-core input maps."""
    cosT = np.concatenate([freqs_cos.T, freqs_cos.T], 0)  # [128, S]
    sinT = np.concatenate([freqs_sin.T, -freqs_sin.T], 0)
    cc = _bf(cosT)
    ss = _bf(sinT)
    wqT = Wq.T    # [H, H]
    wkT = Wk.T    # [H, 512]
    wvT = Wv.T
    woT = Wo.T    # [H, H]
    h8 = []
    for b in range(B):
        hi, lo = _split8(hidden_states[b].T * np.float32(HS))
        h8.append((_pack_h(hi), _pack_h(lo)))
    in_maps = []
    for c in range(NC):
        b, g = divmod(c, HQ)
        cols = np.concatenate(
            [(4 * g + m) * HD + _RP for m in range(HQ)])      # [512]
        wqh, wql = _split8(wqT[:, cols] * np.float32(WS))
        # head-major packing: [p, m, kp, i, j]
        wqh = np.concatenate(
            [_pack_w(wqh[:, m * 128:(m + 1) * 128]) for m in range(HQ)], 1)
        wql = np.concatenate(
            [_pack_w(wql[:, m * 128:(m + 1) * 128]) for m in range(HQ)], 1)
        wkh, wkl = _split8(wkT[:, g * HD + _RP] * np.float32(WS))
        wvh, wvl = _split8(wvT[:, g * HD: (g + 1) * HD] * np.float32(WS))
        wo_p = (woT[4 * g * HD: (4 * g + 4) * HD, :] /
                np.float32(WS * HS)).reshape(HQ, 128, H).transpose(1, 0, 2)
        in_maps.append({
            "h8h": h8[b][0], "h8l": h8[b][1],
            "wq8h": np.ascontiguousarray(wqh),
            "wq8l": np.ascontiguousarray(wql),
            "wk8h": _pack_w(wkh), "wk8l": _pack_w(wkl),
            "wv8h": _pack_w(wvh), "wv8l": _pack_w(wvl),
            "wo": _bf(wo_p.reshape(128, -1)),
            "cc": cc, "ss": ss,
        })
    return in_maps


_CACHE = {}


def _get_nc(reps=1):
    if reps not in _CACHE:
        _CACHE[reps] = build(reps)
    return _CACHE[reps]


def kernel(hidden_states, freqs_cos, freqs_sin, Wq, Wk, Wv, Wo):
    in_maps = prep_inputs(
        np.asarray(hidden_states, np.float32),
        np.asarray(freqs_cos, np.float32),
        np.asarray(freqs_sin, np.float32),
        np.asarray(Wq, np.float32),
        np.asarray(Wk, np.float32),
        np.asarray(Wv, np.float32),
        np.asarray(Wo, np.float32),
    )
    nc = _get_nc(int(os.environ.get("KERNEL_REPS", "1")))
    res = run_bass_kernel_spmd(nc, in_maps, core_ids=list(range(NC)))
    out = np.zeros((B, S, H), np.float32)
    for c in range(NC):
        b = c // HQ
        out[b] += np.asarray(res.results[c]["out"], np.float32)
    return out
